# revision 13
# baseline (speedup 1.0000x reference)
"""Trainium2 Bass/Tile kernel for nn_CNN_77077483094746.

Single tiny sample (x: [1,1,18,140]) -> (1,2). The whole forward pass runs on
one NeuronCore; the same program is executed SPMD on all 8 cores (identical
inputs), output taken from core 0.

Strategy: everything that depends only on the WEIGHTS is precomputed on the
host in numpy (f64) and shipped as two packed constant tensors laid out
exactly as SBUF wants them:
  - cb (bf16, [128, XB]): all matmul operands — pre-transposed weights, the
    q/k projections folded into single Gram matrices G = Wq^T Wk (so
    S = [eeg,1] @ (Gpack @ kA^T + u) per branch, 2 matmuls instead of 3 and
    no weight transposes on device), block-diagonal packs for the four
    cross-modal branches (their q/k/v/out projections each become ONE
    matmul), the block-diagonal conv weights, fc weights.
  - cf (f32, [128, 128]): per-partition bias columns (bias folds: the value
    bias is folded into the output projection via softmax row-sums == 1).

The device program is ~50 LDWEIGHTS+MATMUL pairs, all bf16 (1 PE cycle/row),
5 input DMAs total. Only x-dependent compute runs on device. exp() is taken
without max-subtraction (|S| <~ 10 by construction, f32 range is safe), so
softmax is exp + accum rowsum + reciprocal + scale. Sigmoids use
1/(1+exp(-z)) on the already-loaded Exp table (no activation-table swap:
ReLU+bias and max-pool run on DVE).

Engine streams are emitted interleaved (A/B stage-1 chains, 4-way stage-2
branches) so PE never head-of-line blocks on DVE/ACT post-processing.
"""
import math
from contextlib import ExitStack

import numpy as np
import ml_dtypes

import concourse.bass as bass
import concourse.mybir as mybir
import concourse.tile as tile
from concourse import bacc
from concourse.bass_utils import run_bass_kernel_spmd
from concourse.masks import make_identity

WL = 140
OFC = 118
TDN = 21
D_CM = 16
N_BR = 4
C_OUT = 10
KS = 9
NCONV = OFC - KS + 1
F32 = mybir.dt.float32
BF16 = mybir.dt.bfloat16
N_CORES = 8

XB = 1536
XF = 256

# cb column layout (bf16 pack)
C_GPA = 0        # GpackA [118, 119]
C_GPB = 119      # GpackB [118, 119]
C_WVA = 238      # WvT_A  [118, 118]
C_WVB = 356      # WvT_B  [118, 118]
C_OWA = 474      # owT_A  [118, 118]
C_OWB = 592      # owT_B  [118, 118]
C_HP = 710       # per-branch Hpack_i [16, 17] at C_HP+17i
C_VPBD = 778     # vpbd   [80, 64] (kv blocks at rows 0/32/64)
C_WOBD = 842     # per-branch WO_i [16, 64] at C_WOBD+64i
C_CONV = 1098    # convwT [64, 9*40]
C_FC1 = 1458     # fc1T   [40, 40]
C_FC2 = 1498     # fc2T   [40, 2]
C_PROJ = 1500    # projvec [1, 32]
C_ONES = 1532    # ones16 [16, 1]

# cf column layout (f32 pack)
F_UCA = 0        # ucol_A [119, 1]
F_UCB = 1        # ucol_B [119, 1]
F_OB16A = 2      # 16*obrow_A [118, 1]
F_OB16B = 3      # 16*obrow_B [118, 1]
F_OBROW_A = 4    # obrow_A at row 0: [1, 118]
F_OBROW_B = 128  # obrow_B at row 0: [1, 118]
F_U2 = 122       # per-branch u2col_i [17, 1] at F_U2+i (4 cols)
F_CONVB = 126    # convb [40, 1]
F_NFB1 = 127     # negfb1 [40, 1]
F_OB2 = 246      # ob2 [64, 1]
F_NFB2 = 247     # negfb2 [2, 1]

INPUT_SPECS = {
    "x": ((1, 1, 18, WL), F32),
    "cb": ((128, XB), BF16),
    "cf": ((128, XF), F32),
}


def pack_consts(inp):
    """Host: all weight-only transforms, computed in f64."""
    s1 = 1.0 / math.sqrt(OFC)
    sb = 1.0 / math.sqrt(D_CM)
    cb = np.zeros((128, XB), np.float64)
    cf = np.zeros((128, XF), np.float64)
    for br, pre, cg, cwv, cow, fuc, fob16, obrow_c in (
            ("A", "tdA", C_GPA, C_WVA, C_OWA, F_UCA, F_OB16A, F_OBROW_A),
            ("B", "tdB", C_GPB, C_WVB, C_OWB, F_UCB, F_OB16B, F_OBROW_B)):
        in_w = np.asarray(inp[f"{pre}_in_w"], np.float64)
        in_b = np.asarray(inp[f"{pre}_in_b"], np.float64)
        out_w = np.asarray(inp[f"{pre}_out_w"], np.float64)
        out_b = np.asarray(inp[f"{pre}_out_b"], np.float64)
        wq, wk, wv = in_w[0:OFC], in_w[OFC:2*OFC], in_w[2*OFC:3*OFC]
        bq, bk, bv = in_b[0:OFC], in_b[OFC:2*OFC], in_b[2*OFC:3*OFC]
        cb[0:OFC, cg:cg+OFC] = s1 * (wq.T @ wk).T          # Gpack[j, i]=s1*G[i,j]
        cb[0:OFC, cg+OFC] = s1 * (wk.T @ bq)               # v-row
        cb[0:OFC, cwv:cwv+OFC] = wv.T
        cb[0:OFC, cow:cow+OFC] = out_w.T
        cf[0:OFC, fuc] = s1 * (wq.T @ bk)
        cf[OFC, fuc] = s1 * (bq @ bk)
        obr = out_w @ bv + out_b
        cf[0:OFC, fob16] = 16.0 * obr
        cf[0, obrow_c:obrow_c+OFC] = obr
    cb[0, C_PROJ:C_PROJ+16] = np.asarray(inp["projA_w"], np.float64)[:, 0]
    cb[0, C_PROJ+16:C_PROJ+32] = np.asarray(inp["projB_w"], np.float64)[:, 0]
    cb[0:16, C_ONES] = 1.0

    cm_in_w = np.asarray(inp["cm_in_w"], np.float64)
    cm_in_b = np.asarray(inp["cm_in_b"], np.float64)
    cm_out_w = np.asarray(inp["cm_out_w"], np.float64)
    cm_out_b = np.asarray(inp["cm_out_b"], np.float64)
    # KV row-block base per branch (kv = [eeg, wA, wB, eeg]; blocks at
    # partition bases 0/32/64, branch 3 reuses the eeg block at 0)
    kvb = [0, 32, 64, 0]
    for i in range(N_BR):
        wq, wk, wv = (cm_in_w[i, 0:D_CM], cm_in_w[i, D_CM:2*D_CM],
                      cm_in_w[i, 2*D_CM:3*D_CM])
        bq, bk, bv = (cm_in_b[i, 0:D_CM], cm_in_b[i, D_CM:2*D_CM],
                      cm_in_b[i, 2*D_CM:3*D_CM])
        cb[kvb[i]:kvb[i]+16, C_HP+17*i:C_HP+17*i+16] = sb * (wq.T @ wk).T
        cb[kvb[i]:kvb[i]+16, C_HP+17*i+16] = sb * (wk.T @ bq)
        cf[0:16, F_U2+i] = sb * (wq.T @ bk)
        cf[16, F_U2+i] = sb * (bq @ bk)
        cb[kvb[i]:kvb[i]+16, C_VPBD+16*i:C_VPBD+16*i+16] = wv.T
        cb[0:16, C_WOBD+64*i+16*i:C_WOBD+64*i+16*i+16] = cm_out_w[i].T
        cf[16*i:16*i+16, F_OB2] = cm_out_w[i] @ bv + cm_out_b[i]
    cw = np.asarray(inp["conv_w"], np.float64)
    for i in range(N_BR):
        # convwT[16i+c, k, 10i+oc] = conv_w[i, oc, c, k]
        for k in range(KS):
            cb[16*i:16*i+16, C_CONV+40*k+10*i:C_CONV+40*k+10*i+10] = cw[i, :, :, k].T
    cf[0:40, F_CONVB] = np.asarray(inp["conv_b"], np.float64).reshape(40)
    cb[0:40, C_FC1:C_FC1+40] = np.asarray(inp["fc1_w"], np.float64).T
    cf[0:40, F_NFB1] = -np.asarray(inp["fc1_b"], np.float64)
    cb[0:40, C_FC2:C_FC2+2] = np.asarray(inp["fc2_w"], np.float64).T
    cf[0:2, F_NFB2] = -np.asarray(inp["fc2_b"], np.float64)
    return (np.ascontiguousarray(cb.astype(ml_dtypes.bfloat16)),
            np.ascontiguousarray(cf.astype(np.float32)))


def pack_inputs(inputs):
    cb, cf = pack_consts(inputs)
    x = np.ascontiguousarray(np.asarray(inputs["x"]), dtype=np.float32)
    return {"x": x, "cb": cb, "cf": cf}


def _emit(nc, tc, H, out_ap):
    AF = mybir.ActivationFunctionType
    ALU = mybir.AluOpType
    X = mybir.AxisListType.X

    ctx = ExitStack()
    consts = ctx.enter_context(tc.tile_pool(name="consts", bufs=1))
    work = ctx.enter_context(tc.tile_pool(name="work", bufs=1))
    psum = ctx.enter_context(tc.tile_pool(name="psum", bufs=1, space="PSUM"))

    def dram_ap(handle, off, dims):
        return bass.AP(tensor=handle, offset=off, ap=[list(d) for d in dims])

    def pst(shape, nm, tag, bufs=2, dt=F32):
        return psum.tile(shape, dt, name=nm, tag=tag, bufs=bufs)

    x_h = H["x"]

    # ============================ DMA issue ===============================
    # Few fragments per DMA: per-DMA latency is ~2.2us + ~6.5ns/fragment.
    NB1 = C_WVA          # Gpack A/B
    NB2 = C_HP - NB1     # WvT + owT
    NB3 = XB - C_HP      # stage-2 packs + conv/fc tail
    cb1_sb = consts.tile([128, NB1], BF16, name="cb1")
    cb2_sb = consts.tile([128, NB2], BF16, name="cb2")
    cb3_sb = consts.tile([128, NB3], BF16, name="cb3")
    cf_sb = consts.tile([128, XF], F32, name="cf")
    kab_nat = work.tile([TDN, 2, OFC], F32, name="kab_nat")
    eeg_raw = work.tile([16, OFC], F32, name="eeg_raw")

    # SP: Gpack chunk first (gates the first matmul), then WvT/owT chunk,
    # then kab (contiguous 472B rows, 42 fragments)
    nc.sync.dma_start(out=cb1_sb[:, :], in_=dram_ap(H["cb"], 0, [(XB, 128), (1, NB1)]))
    nc.sync.dma_start(out=kab_nat[:, :, :],
                      in_=dram_ap(x_h, 0, [(1, TDN), (17 * WL, 2), (1, OFC)]))
    nc.sync.dma_start(out=cb2_sb[:, :],
                      in_=dram_ap(H["cb"], NB1, [(XB, 128), (1, NB2)]))
    # ACT: eeg rows first, then the late-phase const chunk
    nc.scalar.dma_start(out=eeg_raw[:, :],
                        in_=dram_ap(x_h, WL + (WL - OFC), [(WL, 16), (1, OFC)]))
    nc.scalar.dma_start(out=cb3_sb[:, :],
                        in_=dram_ap(H["cb"], C_HP, [(XB, 128), (1, NB3)]))
    # gpsimd SWDGE: f32 bias pack (first consumed ~10.5us in)
    nc.gpsimd.dma_start(out=cf_sb[:, :], in_=dram_ap(H["cf"], 0, [(XF, 128), (1, XF)]))

    # ======================= early prep (gpsimd/DVE) ======================
    identity = consts.tile([128, 128], BF16, name="identity")
    make_identity(nc, identity)

    eeg_ext = work.tile([16, OFC + 1], BF16, name="eeg_ext")
    nc.gpsimd.memset(eeg_ext[:, OFC:OFC+1], 1.0)

    dataA = work.tile([17, OFC], BF16, name="dataA")
    dataE = work.tile([17, OFC], BF16, name="dataE")
    dataB = work.tile([17, OFC], BF16, name="dataB")
    KV = work.tile([80, OFC], BF16, name="KV")
    for t in (dataA, dataE, dataB):
        nc.gpsimd.memset(t[:, :], 1.0)  # row 16 stays ones; 0:16 overwritten
    nc.gpsimd.memset(KV[:, :], 0.0)     # gap rows must be 0 for the packs

    kabn_b = work.tile([TDN, 2, OFC], BF16, name="kabn_b")
    nc.vector.tensor_copy(kabn_b[:, :, :], kab_nat[:, :, :])
    kabT_ps = pst([OFC, 2, TDN + 1], "kabT_ps", "s2", bufs=1, dt=BF16)
    nc.tensor.transpose(kabT_ps[:, 0, 0:TDN], kabn_b[:, 0, :], identity[0:TDN, 0:TDN])
    nc.tensor.transpose(kabT_ps[:, 1, 0:TDN], kabn_b[:, 1, :], identity[0:TDN, 0:TDN])
    kab_b = work.tile([OFC, 2, TDN], BF16, name="kab_b")
    nc.vector.tensor_copy(kab_b[:, 0, :], kabT_ps[:, 0, 0:TDN])
    nc.vector.tensor_copy(kab_b[:, 1, :], kabT_ps[:, 1, 0:TDN])
    nc.scalar.copy(eeg_ext[:, 0:OFC], eeg_raw[:, :])
    # stage-2 eeg rows (off critical path; Pool engine)
    nc.gpsimd.tensor_copy(KV[0:16, :], eeg_raw[:, :])
    nc.gpsimd.tensor_copy(dataE[0:16, :], eeg_raw[:, :])

    # eeg^T (with ones row 118) via PE transpose
    eegT_ps = pst([OFC + 1, 16], "eegT_ps", "c", dt=BF16)
    nc.tensor.transpose(eegT_ps[:, :], eeg_ext[:, :], identity[0:16, 0:16])
    eegT = work.tile([OFC + 1, 16], BF16, name="eegT")
    nc.vector.tensor_copy(eegT[:, :], eegT_ps[:, :])

    # ===================== stage 1 (A/B interleaved) ======================
    GP = {"A": cb1_sb[0:OFC, C_GPA:C_GPA+OFC+1],
          "B": cb1_sb[0:OFC, C_GPB:C_GPB+OFC+1]}
    WV = {"A": cb2_sb[0:OFC, C_WVA-NB1:C_WVA-NB1+OFC],
          "B": cb2_sb[0:OFC, C_WVB-NB1:C_WVB-NB1+OFC]}
    OW = {"A": cb2_sb[0:OFC, C_OWA-NB1:C_OWA-NB1+OFC],
          "B": cb2_sb[0:OFC, C_OWB-NB1:C_OWB-NB1+OFC]}
    UC = {"A": cf_sb[0:OFC+1, F_UCA:F_UCA+1], "B": cf_sb[0:OFC+1, F_UCB:F_UCB+1]}
    OB16 = {"A": cf_sb[0:OFC, F_OB16A:F_OB16A+1], "B": cf_sb[0:OFC, F_OB16B:F_OB16B+1]}
    OBROW = {"A": cf_sb[0:1, F_OBROW_A:F_OBROW_A+OFC],
             "B": cf_sb[0:1, F_OBROW_B:F_OBROW_B+OFC]}
    PROJ = {"A": cb3_sb[0:1, C_PROJ-C_HP:C_PROJ-C_HP+16],
            "B": cb3_sb[0:1, C_PROJ-C_HP+16:C_PROJ-C_HP+32]}
    kT = {"A": kab_b[:, 0, :], "B": kab_b[:, 1, :]}
    tag1 = {"A": "a", "B": "b"}
    cpe = {"A": nc.vector, "B": nc.scalar}  # PSUM->SBUF copy engine per branch

    def cp(eng, out, in_):
        (eng.tensor_copy if eng is nc.vector else eng.copy)(out, in_)

    def cpadd(eng, out, in_, bias):
        if eng is nc.vector:
            eng.tensor_scalar_add(out, in_, bias)
        else:
            eng.add(out, in_, bias)

    s1 = {"A": {}, "B": {}}

    def ps1(br, shape, nm):
        return pst(shape, f"{nm}_{br}", tag1[br])

    def gk_mm(br):
        d = s1[br]
        d["gk_ps"] = ps1(br, [OFC + 1, TDN], "gk")
        nc.tensor.matmul(d["gk_ps"][:, :], GP[br], kT[br])

    def gk_post(br):
        d = s1[br]
        d["gk"] = work.tile([OFC + 1, TDN], BF16, name=f"gk_{br}")
        cpadd(cpe[br], d["gk"][:, :], d["gk_ps"][:, :], UC[br])

    def vp_mm(br):
        d = s1[br]
        d["vp_ps"] = ps1(br, [TDN, OFC], "vp")
        nc.tensor.matmul(d["vp_ps"][:, :], kT[br], WV[br])

    def vp_post(br):
        d = s1[br]
        d["vp"] = work.tile([TDN, OFC], BF16, name=f"vp_{br}")
        cp(cpe[br], d["vp"][:, :], d["vp_ps"][:, :])

    def s_mm(br):
        d = s1[br]
        d["S_ps"] = ps1(br, [16, TDN], "S")
        nc.tensor.matmul(d["S_ps"][:, :], eegT[:, :], d["gk"][:, :])

    def softmax1(br):
        d = s1[br]
        d["P"] = work.tile([16, TDN], BF16, name=f"P_{br}")
        nc.scalar.activation(d["P"][:, :], d["S_ps"][:, :], AF.Exp)

    def rinv1(br):
        # off the critical path: attnT/ZT/att use unnormalized P; the
        # normalization lands on the att PSUM->SBUF copy (per-q scale)
        d = s1[br]
        d["rowsum"] = work.tile([16, 1], F32, name=f"rowsum_{br}")
        nc.vector.reduce_sum(d["rowsum"][:, :], d["P"][:, :], axis=X)
        d["rinv"] = work.tile([16, 1], F32, name=f"rinv_{br}")
        nc.vector.reciprocal(d["rinv"][:, :], d["rowsum"][:, :])

    def attnT_t(br):
        d = s1[br]
        d["aT_ps"] = pst([TDN, 16], f"aT_{br}", tag1[br], dt=BF16)
        nc.tensor.transpose(d["aT_ps"][:, :], d["P"][:, :], identity[0:16, 0:16])

    def attnT_cp(br):
        d = s1[br]
        d["aT"] = work.tile([TDN, 16], BF16, name=f"aT_{br}")
        cp(cpe[br], d["aT"][:, :], d["aT_ps"][:, :])

    def zt_mm(br):
        d = s1[br]
        d["ZT_ps"] = ps1(br, [OFC, 16], "ZT")
        nc.tensor.matmul(d["ZT_ps"][:, :], d["vp"][:, :], d["aT"][:, :])

    def zt_cp(br):
        d = s1[br]
        d["ZT"] = work.tile([OFC, 16], BF16, name=f"ZT_{br}")
        cp(cpe[br], d["ZT"][:, :], d["ZT_ps"][:, :])

    def att_mm(br):
        d = s1[br]
        d["att_ps"] = ps1(br, [16, OFC], "att")
        nc.tensor.matmul(d["att_ps"][:, :], d["ZT"][:, :], OW[br])

    def att_cp(br):
        d = s1[br]
        d["att"] = work.tile([16, OFC], BF16, name=f"att_{br}")
        if br == "A":
            nc.vector.tensor_scalar_mul(d["att"][:, :], d["att_ps"][:, :],
                                        d["rinv"][:, :])
        else:
            nc.scalar.activation(d["att"][:, :], d["att_ps"][:, :], AF.Copy,
                                 scale=d["rinv"][:, :])

    def svec_mm(br):
        d = s1[br]
        d["svec_ps"] = ps1(br, [OFC, 1], "svec")
        nc.tensor.matmul(d["svec_ps"][:, :], d["att"][:, :],
                         cb3_sb[0:16, C_ONES-C_HP:C_ONES-C_HP+1])

    def svec_post(br):
        d = s1[br]
        d["svec"] = work.tile([OFC, 1], BF16, name=f"svec_{br}")
        cpadd(cpe[br], d["svec"][:, :], d["svec_ps"][:, :], OB16[br])

    def sc_mm(br):
        d = s1[br]
        d["sc_ps"] = ps1(br, [1, 16], "sc")
        nc.tensor.matmul(d["sc_ps"][:, :], d["svec"][:, :], eegT[0:OFC, :])

    def sel_post(br):
        d = s1[br]
        d["m"] = work.tile([1, 1], F32, name=f"m_{br}")
        nc.vector.reduce_max(d["m"][:, :], d["sc_ps"][:, :], axis=X)
        d["ohr"] = work.tile([1, 16], BF16, name=f"ohr_{br}")
        nc.vector.tensor_scalar(d["ohr"][:, :], d["sc_ps"][:, :], d["m"][:, :],
                                None, op0=ALU.is_equal)

    def oh_t(br):
        d = s1[br]
        d["oh_ps"] = pst([16, 1], f"oh_{br}", tag1[br], dt=BF16)
        nc.tensor.transpose(d["oh_ps"][:, :], d["ohr"][:, :], identity[0:1, 0:1])

    def oh_cp(br):
        d = s1[br]
        d["oh"] = work.tile([16, 1], BF16, name=f"oh_{br}")
        cp(cpe[br], d["oh"][:, :], d["oh_ps"][:, :])

    def row_mm(br):
        d = s1[br]
        d["row_ps"] = ps1(br, [1, OFC], "row")
        nc.tensor.matmul(d["row_ps"][:, :], d["oh"][:, :], d["att"][:, :])

    def row_post(br):
        d = s1[br]
        d["row"] = work.tile([1, OFC], BF16, name=f"row_{br}")
        nc.vector.tensor_add(d["row"][:, :], d["row_ps"][:, :], OBROW[br])

    def w_mm(br):
        d = s1[br]
        d["w_ps"] = ps1(br, [16, OFC], "w")
        nc.tensor.matmul(d["w_ps"][:, :], PROJ[br], d["row"][:, :])

    def w_cp(br):
        # wA -> DATA rows 0:16 and KV rows 16:32; wB -> DATA 51:67, KV 32:48
        d = s1[br]
        if br == "A":
            nc.vector.tensor_copy(dataA[0:16, :], d["w_ps"][:, :])
            nc.scalar.copy(KV[32:48, :], d["w_ps"][:, :])
        else:
            nc.vector.tensor_copy(dataB[0:16, :], d["w_ps"][:, :])
            nc.scalar.copy(KV[64:80, :], d["w_ps"][:, :])

    gk_mm("A")
    gk_mm("B")
    gk_post("A")
    vp_mm("A")
    gk_post("B")
    vp_mm("B")
    s_mm("A")
    vp_post("A")
    s_mm("B")
    softmax1("A")
    vp_post("B")
    rinv1("A")
    softmax1("B")
    attnT_t("A")
    attnT_cp("A")
    rinv1("B")
    zt_mm("A")
    attnT_t("B")
    zt_cp("A")
    attnT_cp("B")
    att_mm("A")
    zt_mm("B")
    att_cp("A")
    zt_cp("B")
    svec_mm("A")
    att_mm("B")
    svec_post("A")
    att_cp("B")
    sc_mm("A")
    svec_mm("B")
    sel_post("A")
    svec_post("B")
    oh_t("A")
    sc_mm("B")
    oh_cp("A")
    sel_post("B")
    row_mm("A")
    oh_t("B")
    row_post("A")
    oh_cp("B")
    w_mm("A")
    row_mm("B")
    w_cp("A")
    row_post("B")
    w_mm("B")
    w_cp("B")

    # ===================== stage 2 (4-way lockstep) =======================
    cpe2 = [nc.vector, nc.scalar, nc.vector, nc.scalar]
    kvb = [0, 32, 64, 0]
    hx_ps = [pst([D_CM + 1, OFC], f"hx_ps_{i}", "c" if i % 2 == 0 else "d",
                 bufs=2 if i % 2 == 0 else 1) for i in range(N_BR)]
    for i in range(N_BR):
        nc.tensor.matmul(hx_ps[i][:, :],
                         cb3_sb[kvb[i]:kvb[i]+16, 17*i:17*i+17],
                         KV[kvb[i]:kvb[i]+16, :])
    vp2_ps = pst([OFC, 64], "vp2_ps", "d", bufs=1)
    nc.tensor.matmul(vp2_ps[:, :], KV[:, :],
                     cb3_sb[0:80, C_VPBD-C_HP:C_VPBD-C_HP+64])
    hx = [work.tile([D_CM + 1, OFC], BF16, name=f"hx_{i}") for i in range(N_BR)]
    for i in range(N_BR):
        cpadd(cpe2[i], hx[i][:, :], hx_ps[i][:, :], cf_sb[0:17, F_U2+i:F_U2+i+1])
    vp2 = work.tile([OFC, 64], BF16, name="vp2")
    nc.scalar.copy(vp2[:, :], vp2_ps[:, :])

    data2 = [dataA, dataE, dataE, dataB]
    S2_ps = pst([OFC, N_BR * OFC], "S2_ps", "s2", bufs=1)
    for i in range(N_BR):
        nc.tensor.matmul(S2_ps[:, OFC*i:OFC*(i+1)], data2[i][:, :], hx[i][:, :])

    b = [dict() for _ in range(N_BR)]
    pne = [nc.vector, nc.gpsimd, nc.vector, nc.gpsimd]
    for i in range(N_BR):
        b[i]["P"] = work.tile([OFC, OFC], BF16, name=f"P2_{i}")
        nc.scalar.activation(b[i]["P"][:, :], S2_ps[:, OFC*i:OFC*(i+1)], AF.Exp)
        b[i]["rs"] = work.tile([OFC, 1], F32, name=f"rs2_{i}")
        nc.vector.reduce_sum(b[i]["rs"][:, :], b[i]["P"][:, :], axis=X)
        b[i]["rinv"] = work.tile([OFC, 1], F32, name=f"rinv2_{i}")
        nc.vector.reciprocal(b[i]["rinv"][:, :], b[i]["rs"][:, :])
        b[i]["Pn"] = work.tile([OFC, OFC], BF16, name=f"Pn2_{i}")
        pne[i].tensor_scalar_mul(b[i]["Pn"][:, :], b[i]["P"][:, :],
                                 b[i]["rinv"][:, :])
        b[i]["aT_ps"] = pst([OFC, OFC], f"aT2_{i}", "a" if i % 2 == 0 else "b",
                            dt=BF16)
        nc.tensor.transpose(b[i]["aT_ps"][:, :], b[i]["Pn"][:, :],
                            identity[0:OFC, 0:OFC])
        b[i]["aT"] = work.tile([OFC, OFC], BF16, name=f"aT2_{i}")
        cp(cpe2[i], b[i]["aT"][:, :], b[i]["aT_ps"][:, :])

    oT_ps = pst([64, OFC], "oT_ps", "s2", bufs=1)
    for i in range(N_BR):
        b[i]["ZT_ps"] = pst([D_CM, OFC], f"ZT2_{i}", "c" if i % 2 == 0 else "d",
                            bufs=2 if i % 2 == 0 else 1)
        nc.tensor.matmul(b[i]["ZT_ps"][:, :], vp2[:, 16*i:16*(i+1)],
                         b[i]["aT"][:, :])
        b[i]["zt"] = work.tile([D_CM, OFC], BF16, name=f"zt_{i}")
        cp(cpe2[i], b[i]["zt"][:, :], b[i]["ZT_ps"][:, :])
    for i in range(N_BR):
        nc.tensor.matmul(oT_ps[:, :],
                         cb3_sb[0:16, C_WOBD-C_HP+64*i:C_WOBD-C_HP+64*i+64],
                         b[i]["zt"][:, :], start=(i == 0), stop=(i == N_BR - 1))
    oTall = work.tile([64, OFC], BF16, name="oTall")
    nc.vector.tensor_scalar_add(oTall[:, :], oT_ps[:, :], cf_sb[0:64, F_OB2:F_OB2+1])

    # ======================== conv + classifier ===========================
    y_ps = pst([4 * C_OUT, NCONV], "y_ps", "d", bufs=1)
    for k in range(KS):
        nc.tensor.matmul(y_ps[:, :],
                         cb3_sb[0:64, C_CONV-C_HP+40*k:C_CONV-C_HP+40*k+40],
                         oTall[:, k:k+NCONV], start=(k == 0), stop=(k == KS - 1))
    relu = work.tile([4 * C_OUT, NCONV], F32, name="relu")
    nc.vector.tensor_scalar(relu[:, :], y_ps[:, :], cf_sb[0:40, F_CONVB:F_CONVB+1],
                            0.0, op0=ALU.add, op1=ALU.max)
    feat = work.tile([4 * C_OUT, 1], BF16, name="feat")
    nc.vector.reduce_max(feat[:, :], relu[:, :], axis=X)

    h_ps = pst([40, 1], "h_ps", "a")
    nc.tensor.matmul(h_ps[:, :], cb3_sb[0:40, C_FC1-C_HP:C_FC1-C_HP+40],
                     feat[:, :])
    eh = work.tile([40, 1], F32, name="eh")
    nc.scalar.activation(eh[:, :], h_ps[:, :], AF.Exp,
                         bias=cf_sb[0:40, F_NFB1:F_NFB1+1], scale=-1.0)
    eh1 = work.tile([40, 1], F32, name="eh1")
    nc.scalar.add(eh1[:, :], eh[:, :], 1.0)
    h = work.tile([40, 1], BF16, name="h")
    with nc.allow_low_precision(reason="bf16 operand for the 2x40 head matmul"):
        nc.vector.reciprocal(h[:, :], eh1[:, :])

    o_ps = pst([2, 1], "o_ps", "d", bufs=1)
    nc.tensor.matmul(o_ps[:, :], cb3_sb[0:40, C_FC2-C_HP:C_FC2-C_HP+2],
                     h[:, :])
    eo = work.tile([2, 1], F32, name="eo")
    nc.scalar.activation(eo[:, :], o_ps[:, :], AF.Exp,
                         bias=cf_sb[0:2, F_NFB2:F_NFB2+1], scale=-1.0)
    eo1 = work.tile([2, 1], F32, name="eo1")
    nc.scalar.add(eo1[:, :], eo[:, :], 1.0)
    res = work.tile([2, 1], F32, name="res")
    nc.vector.reciprocal(res[:, :], eo1[:, :])

    nc.sync.dma_start(out=out_ap, in_=res[:, :])
    ctx.close()


_CACHE = {}


def build(debug_taps=False):
    key = ("nc", debug_taps)
    if key in _CACHE:
        return _CACHE[key]
    nc = bacc.Bacc("TRN2", target_bir_lowering=False, debug=False,
                   num_devices=N_CORES, num_swdge_queues=4,
                   dynamic_dma_scratch_size=65536)
    H = {name: nc.dram_tensor(name, list(shape), dt, kind="ExternalInput")
         for name, (shape, dt) in INPUT_SPECS.items()}
    out_t = nc.dram_tensor("out", [1, 2], F32, kind="ExternalOutput")
    with tile.TileContext(nc) as tc:
        _emit(nc, tc, H, out_t.ap())
    nc.compile()
    _CACHE[key] = nc
    return nc


def kernel(**inputs):
    nc = build()
    in_map = pack_inputs(inputs)
    res = run_bass_kernel_spmd(nc, [in_map] * N_CORES,
                               core_ids=list(range(N_CORES)))
    return res.results[0]["out"]


# revision 14
# speedup vs baseline: 1.1138x; 1.1138x over previous
"""Trainium2 Bass/Tile kernel for nn_CNN_77077483094746.

Single tiny sample (x: [1,1,18,140]) -> (1,2). The whole forward pass runs on
one NeuronCore; the same program is executed SPMD on all 8 cores (identical
inputs), output taken from core 0.

Strategy: everything that depends only on the WEIGHTS is precomputed on the
host in numpy (f64) and shipped as two packed constant tensors laid out
exactly as SBUF wants them:
  - cb (bf16, [128, XB]): all matmul operands — pre-transposed weights, the
    q/k projections folded into single Gram matrices G = Wq^T Wk (so
    S = [eeg,1] @ (Gpack @ kA^T + u) per branch, 2 matmuls instead of 3 and
    no weight transposes on device), block-diagonal packs for the four
    cross-modal branches (their q/k/v/out projections each become ONE
    matmul), the block-diagonal conv weights, fc weights.
  - cf (f32, [128, 128]): per-partition bias columns (bias folds: the value
    bias is folded into the output projection via softmax row-sums == 1).

The device program is ~50 LDWEIGHTS+MATMUL pairs, all bf16 (1 PE cycle/row),
5 input DMAs total. Only x-dependent compute runs on device. exp() is taken
without max-subtraction (|S| <~ 10 by construction, f32 range is safe), so
softmax is exp + accum rowsum + reciprocal + scale. Sigmoids use
1/(1+exp(-z)) on the already-loaded Exp table (no activation-table swap:
ReLU+bias and max-pool run on DVE).

Engine streams are emitted interleaved (A/B stage-1 chains, 4-way stage-2
branches) so PE never head-of-line blocks on DVE/ACT post-processing.
"""
import math
from contextlib import ExitStack

import numpy as np
import ml_dtypes

import concourse.bass as bass
import concourse.mybir as mybir
import concourse.tile as tile
from concourse import bacc
from concourse.bass_utils import run_bass_kernel_spmd
from concourse.masks import make_identity

WL = 140
OFC = 118
TDN = 21
D_CM = 16
N_BR = 4
C_OUT = 10
KS = 9
NCONV = OFC - KS + 1
F32 = mybir.dt.float32
BF16 = mybir.dt.bfloat16
N_CORES = 8

XB = 1536
XF = 256

# cb column layout (bf16 pack)
C_GPA = 0        # GpackA [118, 119]
C_GPB = 119      # GpackB [118, 119]
C_WVA = 238      # WvT_A  [118, 118]
C_WVB = 356      # WvT_B  [118, 118]
C_OWA = 474      # owT_A  [118, 118]
C_OWB = 592      # owT_B  [118, 118]
C_HP = 710       # per-branch Hpack_i [16, 17] at C_HP+17i
C_VPBD = 778     # vpbd   [80, 64] (kv blocks at rows 0/32/64)
C_WOBD = 842     # per-branch WO_i [16, 64] at C_WOBD+64i
C_CONV = 1098    # convwT [64, 9*40]
C_FC1 = 1458     # fc1T   [40, 40]
C_FC2 = 1498     # fc2T   [40, 2]
C_PROJ = 1500    # projvec [1, 32]
C_ONES = 1532    # ones16 [16, 1]

# cf column layout (f32 pack)
F_UCA = 0        # ucol_A [119, 1]
F_UCB = 1        # ucol_B [119, 1]
F_OB16A = 2      # 16*obrow_A [118, 1]
F_OB16B = 3      # 16*obrow_B [118, 1]
F_OBROW_A = 4    # obrow_A at row 0: [1, 118]
F_OBROW_B = 128  # obrow_B at row 0: [1, 118]
F_U2 = 122       # per-branch u2col_i [17, 1] at F_U2+i (4 cols)
F_CONVB = 126    # convb [40, 1]
F_NFB1 = 127     # negfb1 [40, 1]
F_OB2 = 246      # ob2 [64, 1]
F_NFB2 = 247     # negfb2 [2, 1]

INPUT_SPECS = {
    "x": ((1, 1, 18, WL), F32),
    "cb": ((128, XB), BF16),
    "cf": ((128, XF), F32),
}


def pack_consts(inp):
    """Host: all weight-only transforms, computed in f64."""
    s1 = 1.0 / math.sqrt(OFC)
    sb = 1.0 / math.sqrt(D_CM)
    cb = np.zeros((128, XB), np.float64)
    cf = np.zeros((128, XF), np.float64)
    for br, pre, cg, cwv, cow, fuc, fob16, obrow_c in (
            ("A", "tdA", C_GPA, C_WVA, C_OWA, F_UCA, F_OB16A, F_OBROW_A),
            ("B", "tdB", C_GPB, C_WVB, C_OWB, F_UCB, F_OB16B, F_OBROW_B)):
        in_w = np.asarray(inp[f"{pre}_in_w"], np.float64)
        in_b = np.asarray(inp[f"{pre}_in_b"], np.float64)
        out_w = np.asarray(inp[f"{pre}_out_w"], np.float64)
        out_b = np.asarray(inp[f"{pre}_out_b"], np.float64)
        wq, wk, wv = in_w[0:OFC], in_w[OFC:2*OFC], in_w[2*OFC:3*OFC]
        bq, bk, bv = in_b[0:OFC], in_b[OFC:2*OFC], in_b[2*OFC:3*OFC]
        cb[0:OFC, cg:cg+OFC] = s1 * (wq.T @ wk).T          # Gpack[j, i]=s1*G[i,j]
        cb[0:OFC, cg+OFC] = s1 * (wk.T @ bq)               # v-row
        cb[0:OFC, cwv:cwv+OFC] = wv.T
        cb[0:OFC, cow:cow+OFC] = out_w.T
        cf[0:OFC, fuc] = s1 * (wq.T @ bk)
        cf[OFC, fuc] = s1 * (bq @ bk)
        obr = out_w @ bv + out_b
        cf[0:OFC, fob16] = 16.0 * obr
        cf[0, obrow_c:obrow_c+OFC] = obr
    cb[0, C_PROJ:C_PROJ+16] = np.asarray(inp["projA_w"], np.float64)[:, 0]
    cb[0, C_PROJ+16:C_PROJ+32] = np.asarray(inp["projB_w"], np.float64)[:, 0]
    cb[0:16, C_ONES] = 1.0

    cm_in_w = np.asarray(inp["cm_in_w"], np.float64)
    cm_in_b = np.asarray(inp["cm_in_b"], np.float64)
    cm_out_w = np.asarray(inp["cm_out_w"], np.float64)
    cm_out_b = np.asarray(inp["cm_out_b"], np.float64)
    # KV row-block base per branch (kv = [eeg, wA, wB, eeg]; blocks at
    # partition bases 0/32/64, branch 3 reuses the eeg block at 0)
    kvb = [0, 32, 64, 0]
    for i in range(N_BR):
        wq, wk, wv = (cm_in_w[i, 0:D_CM], cm_in_w[i, D_CM:2*D_CM],
                      cm_in_w[i, 2*D_CM:3*D_CM])
        bq, bk, bv = (cm_in_b[i, 0:D_CM], cm_in_b[i, D_CM:2*D_CM],
                      cm_in_b[i, 2*D_CM:3*D_CM])
        cb[kvb[i]:kvb[i]+16, C_HP+17*i:C_HP+17*i+16] = sb * (wq.T @ wk).T
        cb[kvb[i]:kvb[i]+16, C_HP+17*i+16] = sb * (wk.T @ bq)
        cf[0:16, F_U2+i] = sb * (wq.T @ bk)
        cf[16, F_U2+i] = sb * (bq @ bk)
        cb[kvb[i]:kvb[i]+16, C_VPBD+16*i:C_VPBD+16*i+16] = wv.T
        cb[0:16, C_WOBD+64*i+16*i:C_WOBD+64*i+16*i+16] = cm_out_w[i].T
        cf[16*i:16*i+16, F_OB2] = cm_out_w[i] @ bv + cm_out_b[i]
    cw = np.asarray(inp["conv_w"], np.float64)
    for i in range(N_BR):
        # convwT[16i+c, k, 10i+oc] = conv_w[i, oc, c, k]
        for k in range(KS):
            cb[16*i:16*i+16, C_CONV+40*k+10*i:C_CONV+40*k+10*i+10] = cw[i, :, :, k].T
    cf[0:40, F_CONVB] = np.asarray(inp["conv_b"], np.float64).reshape(40)
    cb[0:40, C_FC1:C_FC1+40] = np.asarray(inp["fc1_w"], np.float64).T
    cf[0:40, F_NFB1] = np.asarray(inp["fc1_b"], np.float64)
    cb[0:40, C_FC2:C_FC2+2] = np.asarray(inp["fc2_w"], np.float64).T
    cf[0:2, F_NFB2] = np.asarray(inp["fc2_b"], np.float64)
    return (np.ascontiguousarray(cb.astype(ml_dtypes.bfloat16)),
            np.ascontiguousarray(cf.astype(np.float32)))


def pack_inputs(inputs):
    cb, cf = pack_consts(inputs)
    x = np.ascontiguousarray(np.asarray(inputs["x"]), dtype=np.float32)
    return {"x": x, "cb": cb, "cf": cf}


def _emit(nc, tc, H, out_ap):
    AF = mybir.ActivationFunctionType
    ALU = mybir.AluOpType
    X = mybir.AxisListType.X

    ctx = ExitStack()
    consts = ctx.enter_context(tc.tile_pool(name="consts", bufs=1))
    work = ctx.enter_context(tc.tile_pool(name="work", bufs=1))
    psum = ctx.enter_context(tc.tile_pool(name="psum", bufs=1, space="PSUM"))

    def dram_ap(handle, off, dims):
        return bass.AP(tensor=handle, offset=off, ap=[list(d) for d in dims])

    def pst(shape, nm, tag, bufs=2, dt=F32):
        return psum.tile(shape, dt, name=nm, tag=tag, bufs=bufs)

    x_h = H["x"]

    # ============================ DMA issue ===============================
    # Few fragments per DMA: per-DMA latency is ~2.2us + ~6.5ns/fragment.
    NB1 = C_WVA          # Gpack A/B
    NB2 = C_HP - NB1     # WvT + owT
    NB3 = XB - C_HP      # stage-2 packs + conv/fc tail
    cb1_sb = consts.tile([128, NB1], BF16, name="cb1")
    cb2_sb = consts.tile([128, NB2], BF16, name="cb2")
    cb3_sb = consts.tile([128, NB3], BF16, name="cb3")
    cf_sb = consts.tile([128, XF], F32, name="cf")
    kab_nat = work.tile([TDN, 2, OFC], F32, name="kab_nat")
    eeg_raw = work.tile([16, OFC], F32, name="eeg_raw")

    # gpsimd SWDGE: kab (42 contiguous 472B fragments; arrives first),
    # then the f32 bias pack (first consumed late)
    nc.gpsimd.dma_start(out=kab_nat[:, :, :],
                        in_=dram_ap(x_h, 0, [(1, TDN), (17 * WL, 2), (1, OFC)]))
    nc.gpsimd.dma_start(out=cf_sb[:, :], in_=dram_ap(H["cf"], 0, [(XF, 128), (1, XF)]))
    # SP: Gpack chunk first (gates the first matmul), then WvT/owT
    nc.sync.dma_start(out=cb1_sb[:, :], in_=dram_ap(H["cb"], 0, [(XB, 128), (1, NB1)]))
    nc.sync.dma_start(out=cb2_sb[:, :],
                      in_=dram_ap(H["cb"], NB1, [(XB, 128), (1, NB2)]))
    # ACT: eeg rows first, then the late-phase const chunk
    nc.scalar.dma_start(out=eeg_raw[:, :],
                        in_=dram_ap(x_h, WL + (WL - OFC), [(WL, 16), (1, OFC)]))
    nc.scalar.dma_start(out=cb3_sb[:, :],
                        in_=dram_ap(H["cb"], C_HP, [(XB, 128), (1, NB3)]))

    # ======================= early prep (gpsimd/DVE) ======================
    identity = consts.tile([128, 128], BF16, name="identity")
    make_identity(nc, identity)

    eeg_ext = work.tile([16, OFC + 1], BF16, name="eeg_ext")
    nc.gpsimd.memset(eeg_ext[:, OFC:OFC+1], 1.0)

    dataA = work.tile([17, OFC], BF16, name="dataA")
    dataE = work.tile([17, OFC], BF16, name="dataE")
    dataB = work.tile([17, OFC], BF16, name="dataB")
    KV = work.tile([80, OFC], BF16, name="KV")
    for t in (dataA, dataE, dataB):
        nc.gpsimd.memset(t[:, :], 1.0)  # row 16 stays ones; 0:16 overwritten
    nc.gpsimd.memset(KV[:, :], 0.0)     # gap rows must be 0 for the packs

    kabn_b = work.tile([TDN, 2, OFC], BF16, name="kabn_b")
    nc.vector.tensor_copy(kabn_b[:, :, :], kab_nat[:, :, :])
    kabT_ps = pst([OFC, 2, TDN + 1], "kabT_ps", "s2", bufs=1, dt=BF16)
    nc.tensor.transpose(kabT_ps[:, 0, 0:TDN], kabn_b[:, 0, :], identity[0:TDN, 0:TDN])
    nc.tensor.transpose(kabT_ps[:, 1, 0:TDN], kabn_b[:, 1, :], identity[0:TDN, 0:TDN])
    kab_b = work.tile([OFC, 2, TDN], BF16, name="kab_b")
    nc.vector.tensor_copy(kab_b[:, 0, :], kabT_ps[:, 0, 0:TDN])
    nc.vector.tensor_copy(kab_b[:, 1, :], kabT_ps[:, 1, 0:TDN])
    nc.scalar.copy(eeg_ext[:, 0:OFC], eeg_raw[:, :])
    # stage-2 eeg rows (off critical path; Pool engine)
    nc.gpsimd.tensor_copy(KV[0:16, :], eeg_raw[:, :])
    nc.gpsimd.tensor_copy(dataE[0:16, :], eeg_raw[:, :])

    # eeg^T (with ones row 118) via PE transpose
    eegT_ps = pst([OFC + 1, 16], "eegT_ps", "c", dt=BF16)
    nc.tensor.transpose(eegT_ps[:, :], eeg_ext[:, :], identity[0:16, 0:16])
    eegT = work.tile([OFC + 1, 16], BF16, name="eegT")
    nc.vector.tensor_copy(eegT[:, :], eegT_ps[:, :])

    # ===================== stage 1 (A/B interleaved) ======================
    GP = {"A": cb1_sb[0:OFC, C_GPA:C_GPA+OFC+1],
          "B": cb1_sb[0:OFC, C_GPB:C_GPB+OFC+1]}
    WV = {"A": cb2_sb[0:OFC, C_WVA-NB1:C_WVA-NB1+OFC],
          "B": cb2_sb[0:OFC, C_WVB-NB1:C_WVB-NB1+OFC]}
    OW = {"A": cb2_sb[0:OFC, C_OWA-NB1:C_OWA-NB1+OFC],
          "B": cb2_sb[0:OFC, C_OWB-NB1:C_OWB-NB1+OFC]}
    UC = {"A": cf_sb[0:OFC+1, F_UCA:F_UCA+1], "B": cf_sb[0:OFC+1, F_UCB:F_UCB+1]}
    OB16 = {"A": cf_sb[0:OFC, F_OB16A:F_OB16A+1], "B": cf_sb[0:OFC, F_OB16B:F_OB16B+1]}
    OBROW = {"A": cf_sb[0:1, F_OBROW_A:F_OBROW_A+OFC],
             "B": cf_sb[0:1, F_OBROW_B:F_OBROW_B+OFC]}
    PROJ = {"A": cb3_sb[0:1, C_PROJ-C_HP:C_PROJ-C_HP+16],
            "B": cb3_sb[0:1, C_PROJ-C_HP+16:C_PROJ-C_HP+32]}
    kT = {"A": kab_b[:, 0, :], "B": kab_b[:, 1, :]}
    tag1 = {"A": "a", "B": "b"}
    cpe = {"A": nc.vector, "B": nc.scalar}  # PSUM->SBUF copy engine per branch

    def cp(eng, out, in_):
        (eng.tensor_copy if eng is nc.vector else eng.copy)(out, in_)

    def cpadd(eng, out, in_, bias):
        if eng is nc.vector:
            eng.tensor_scalar_add(out, in_, bias)
        else:
            eng.add(out, in_, bias)

    s1 = {"A": {}, "B": {}}

    def ps1(br, shape, nm):
        return pst(shape, f"{nm}_{br}", tag1[br])

    def gk_mm(br):
        d = s1[br]
        d["gk_ps"] = ps1(br, [OFC + 1, TDN], "gk")
        nc.tensor.matmul(d["gk_ps"][:, :], GP[br], kT[br])

    def gk_post(br):
        d = s1[br]
        d["gk"] = work.tile([OFC + 1, TDN], BF16, name=f"gk_{br}")
        cpadd(cpe[br], d["gk"][:, :], d["gk_ps"][:, :], UC[br])

    def vp_mm(br):
        d = s1[br]
        d["vp_ps"] = ps1(br, [TDN, OFC], "vp")
        nc.tensor.matmul(d["vp_ps"][:, :], kT[br], WV[br])

    def vp_post(br):
        d = s1[br]
        d["vp"] = work.tile([TDN, OFC], BF16, name=f"vp_{br}")
        cp(cpe[br], d["vp"][:, :], d["vp_ps"][:, :])

    def s_mm(br):
        d = s1[br]
        d["S_ps"] = ps1(br, [16, TDN], "S")
        nc.tensor.matmul(d["S_ps"][:, :], eegT[:, :], d["gk"][:, :])

    def softmax1(br):
        d = s1[br]
        d["P"] = work.tile([16, TDN], BF16, name=f"P_{br}")
        nc.scalar.activation(d["P"][:, :], d["S_ps"][:, :], AF.Exp)

    def rinv1(br):
        # off the critical path: attnT/ZT/att use unnormalized P; the
        # normalization lands on the att PSUM->SBUF copy (per-q scale)
        d = s1[br]
        d["rowsum"] = work.tile([16, 1], F32, name=f"rowsum_{br}")
        nc.vector.reduce_sum(d["rowsum"][:, :], d["P"][:, :], axis=X)
        d["rinv"] = work.tile([16, 1], F32, name=f"rinv_{br}")
        nc.vector.reciprocal(d["rinv"][:, :], d["rowsum"][:, :])

    def attnT_t(br):
        d = s1[br]
        d["aT_ps"] = pst([TDN, 16], f"aT_{br}", tag1[br], dt=BF16)
        nc.tensor.transpose(d["aT_ps"][:, :], d["P"][:, :], identity[0:16, 0:16])

    def attnT_cp(br):
        d = s1[br]
        d["aT"] = work.tile([TDN, 16], BF16, name=f"aT_{br}")
        cp(cpe[br], d["aT"][:, :], d["aT_ps"][:, :])

    def zt_mm(br):
        d = s1[br]
        d["ZT_ps"] = ps1(br, [OFC, 16], "ZT")
        nc.tensor.matmul(d["ZT_ps"][:, :], d["vp"][:, :], d["aT"][:, :])

    def zt_cp(br):
        d = s1[br]
        d["ZT"] = work.tile([OFC, 16], BF16, name=f"ZT_{br}")
        cp(cpe[br], d["ZT"][:, :], d["ZT_ps"][:, :])

    def att_mm(br):
        d = s1[br]
        d["att_ps"] = ps1(br, [16, OFC], "att")
        nc.tensor.matmul(d["att_ps"][:, :], d["ZT"][:, :], OW[br])

    def att_cp(br):
        d = s1[br]
        d["att"] = work.tile([16, OFC], BF16, name=f"att_{br}")
        if br == "A":
            nc.vector.tensor_scalar_mul(d["att"][:, :], d["att_ps"][:, :],
                                        d["rinv"][:, :])
        else:
            nc.scalar.activation(d["att"][:, :], d["att_ps"][:, :], AF.Copy,
                                 scale=d["rinv"][:, :])

    def svec_mm(br):
        d = s1[br]
        d["svec_ps"] = ps1(br, [OFC, 1], "svec")
        nc.tensor.matmul(d["svec_ps"][:, :], d["att"][:, :],
                         cb3_sb[0:16, C_ONES-C_HP:C_ONES-C_HP+1])

    def svec_post(br):
        d = s1[br]
        d["svec"] = work.tile([OFC, 1], BF16, name=f"svec_{br}")
        cpadd(cpe[br], d["svec"][:, :], d["svec_ps"][:, :], OB16[br])

    def sc_mm(br):
        d = s1[br]
        d["sc_ps"] = ps1(br, [1, 16], "sc")
        nc.tensor.matmul(d["sc_ps"][:, :], d["svec"][:, :], eegT[0:OFC, :])

    def sel_post(br):
        d = s1[br]
        d["m"] = work.tile([1, 1], F32, name=f"m_{br}")
        nc.vector.reduce_max(d["m"][:, :], d["sc_ps"][:, :], axis=X)
        d["ohr"] = work.tile([1, 16], BF16, name=f"ohr_{br}")
        nc.vector.tensor_scalar(d["ohr"][:, :], d["sc_ps"][:, :], d["m"][:, :],
                                None, op0=ALU.is_equal)

    def oh_t(br):
        d = s1[br]
        d["oh_ps"] = pst([16, 1], f"oh_{br}", tag1[br], dt=BF16)
        nc.tensor.transpose(d["oh_ps"][:, :], d["ohr"][:, :], identity[0:1, 0:1])

    def oh_cp(br):
        d = s1[br]
        d["oh"] = work.tile([16, 1], BF16, name=f"oh_{br}")
        cp(cpe[br], d["oh"][:, :], d["oh_ps"][:, :])

    def row_mm(br):
        d = s1[br]
        d["row_ps"] = ps1(br, [1, OFC], "row")
        nc.tensor.matmul(d["row_ps"][:, :], d["oh"][:, :], d["att"][:, :])

    def row_post(br):
        d = s1[br]
        d["row"] = work.tile([1, OFC], BF16, name=f"row_{br}")
        nc.vector.tensor_add(d["row"][:, :], d["row_ps"][:, :], OBROW[br])

    def w_mm(br):
        d = s1[br]
        d["w_ps"] = ps1(br, [16, OFC], "w")
        nc.tensor.matmul(d["w_ps"][:, :], PROJ[br], d["row"][:, :])

    def w_cp(br):
        # wA -> DATA rows 0:16 and KV rows 16:32; wB -> DATA 51:67, KV 32:48
        d = s1[br]
        if br == "A":
            nc.vector.tensor_copy(dataA[0:16, :], d["w_ps"][:, :])
            nc.scalar.copy(KV[32:48, :], d["w_ps"][:, :])
        else:
            nc.vector.tensor_copy(dataB[0:16, :], d["w_ps"][:, :])
            nc.scalar.copy(KV[64:80, :], d["w_ps"][:, :])

    gk_mm("A")
    gk_mm("B")
    gk_post("A")
    vp_mm("A")
    gk_post("B")
    vp_mm("B")
    s_mm("A")
    vp_post("A")
    s_mm("B")
    softmax1("A")
    vp_post("B")
    rinv1("A")
    softmax1("B")
    attnT_t("A")
    attnT_cp("A")
    rinv1("B")
    zt_mm("A")
    attnT_t("B")
    zt_cp("A")
    attnT_cp("B")
    att_mm("A")
    zt_mm("B")
    att_cp("A")
    zt_cp("B")
    svec_mm("A")
    att_mm("B")
    svec_post("A")
    att_cp("B")
    sc_mm("A")
    svec_mm("B")
    sel_post("A")
    svec_post("B")
    oh_t("A")
    sc_mm("B")
    oh_cp("A")
    sel_post("B")
    row_mm("A")
    oh_t("B")
    row_post("A")
    oh_cp("B")
    w_mm("A")
    row_mm("B")
    w_cp("A")
    row_post("B")
    w_mm("B")
    w_cp("B")

    # ===================== stage 2 (4-way lockstep) =======================
    cpe2 = [nc.vector, nc.scalar, nc.vector, nc.scalar]
    kvb = [0, 32, 64, 0]
    hx_ps = [pst([D_CM + 1, OFC], f"hx_ps_{i}", "c" if i % 2 == 0 else "d",
                 bufs=2 if i % 2 == 0 else 1) for i in range(N_BR)]
    for i in range(N_BR):
        nc.tensor.matmul(hx_ps[i][:, :],
                         cb3_sb[kvb[i]:kvb[i]+16, 17*i:17*i+17],
                         KV[kvb[i]:kvb[i]+16, :])
    vp2_ps = pst([OFC, 64], "vp2_ps", "d", bufs=1)
    nc.tensor.matmul(vp2_ps[:, :], KV[:, :],
                     cb3_sb[0:80, C_VPBD-C_HP:C_VPBD-C_HP+64])
    hx = [work.tile([D_CM + 1, OFC], BF16, name=f"hx_{i}") for i in range(N_BR)]
    for i in range(N_BR):
        cpadd(cpe2[i], hx[i][:, :], hx_ps[i][:, :], cf_sb[0:17, F_U2+i:F_U2+i+1])
    vp2 = work.tile([OFC, 64], BF16, name="vp2")
    nc.scalar.copy(vp2[:, :], vp2_ps[:, :])

    data2 = [dataA, dataE, dataE, dataB]
    S2_ps = pst([OFC, N_BR * OFC], "S2_ps", "s2", bufs=1)
    for i in range(N_BR):
        nc.tensor.matmul(S2_ps[:, OFC*i:OFC*(i+1)], data2[i][:, :], hx[i][:, :])

    b = [dict() for _ in range(N_BR)]
    for i in range(N_BR):
        b[i]["P"] = work.tile([OFC, OFC], BF16, name=f"P2_{i}")
        nc.scalar.activation(b[i]["P"][:, :], S2_ps[:, OFC*i:OFC*(i+1)], AF.Exp)
        b[i]["rs"] = work.tile([OFC, 1], F32, name=f"rs2_{i}")
        nc.vector.reduce_sum(b[i]["rs"][:, :], b[i]["P"][:, :], axis=X)
        b[i]["rinv"] = work.tile([OFC, 1], F32, name=f"rinv2_{i}")
        nc.vector.reciprocal(b[i]["rinv"][:, :], b[i]["rs"][:, :])
        b[i]["Pn"] = work.tile([OFC, OFC], BF16, name=f"Pn2_{i}")
        if i % 2 == 0:
            nc.vector.tensor_scalar_mul(b[i]["Pn"][:, :], b[i]["P"][:, :],
                                        b[i]["rinv"][:, :])
        else:
            nc.scalar.activation(b[i]["Pn"][:, :], b[i]["P"][:, :], AF.Copy,
                                 scale=b[i]["rinv"][:, :])
        b[i]["aT_ps"] = pst([OFC, OFC], f"aT2_{i}", "a" if i % 2 == 0 else "b",
                            dt=BF16)
        nc.tensor.transpose(b[i]["aT_ps"][:, :], b[i]["Pn"][:, :],
                            identity[0:OFC, 0:OFC])
        b[i]["aT"] = work.tile([OFC, OFC], BF16, name=f"aT2_{i}")
        cp(cpe2[i], b[i]["aT"][:, :], b[i]["aT_ps"][:, :])

    oT_ps = pst([64, OFC], "oT_ps", "s2", bufs=1)
    for i in range(N_BR):
        b[i]["ZT_ps"] = pst([D_CM, OFC], f"ZT2_{i}", "c" if i % 2 == 0 else "d",
                            bufs=2 if i % 2 == 0 else 1)
        nc.tensor.matmul(b[i]["ZT_ps"][:, :], vp2[:, 16*i:16*(i+1)],
                         b[i]["aT"][:, :])
        b[i]["zt"] = work.tile([D_CM, OFC], BF16, name=f"zt_{i}")
        cp(cpe2[i], b[i]["zt"][:, :], b[i]["ZT_ps"][:, :])
    for i in range(N_BR):
        nc.tensor.matmul(oT_ps[:, :],
                         cb3_sb[0:16, C_WOBD-C_HP+64*i:C_WOBD-C_HP+64*i+64],
                         b[i]["zt"][:, :], start=(i == 0), stop=(i == N_BR - 1))
    oTall = work.tile([64, OFC], BF16, name="oTall")
    nc.vector.tensor_scalar_add(oTall[:, :], oT_ps[:, :], cf_sb[0:64, F_OB2:F_OB2+1])

    # ======================== conv + classifier ===========================
    y_ps = pst([4 * C_OUT, NCONV], "y_ps", "d", bufs=1)
    for k in range(KS):
        nc.tensor.matmul(y_ps[:, :],
                         cb3_sb[0:64, C_CONV-C_HP+40*k:C_CONV-C_HP+40*k+40],
                         oTall[:, k:k+NCONV], start=(k == 0), stop=(k == KS - 1))
    relu = work.tile([4 * C_OUT, NCONV], F32, name="relu")
    nc.vector.tensor_scalar(relu[:, :], y_ps[:, :], cf_sb[0:40, F_CONVB:F_CONVB+1],
                            0.0, op0=ALU.add, op1=ALU.max)
    feat = work.tile([4 * C_OUT, 1], BF16, name="feat")
    nc.vector.reduce_max(feat[:, :], relu[:, :], axis=X)

    h_ps = pst([40, 1], "h_ps", "a")
    nc.tensor.matmul(h_ps[:, :], cb3_sb[0:40, C_FC1-C_HP:C_FC1-C_HP+40],
                     feat[:, :])
    h = work.tile([40, 1], BF16, name="h")
    # bias holds +fc1_b here (negfb1 slot repurposed on host as +fb)
    nc.scalar.activation(h[:, :], h_ps[:, :], AF.Sigmoid,
                         bias=cf_sb[0:40, F_NFB1:F_NFB1+1], scale=1.0)

    o_ps = pst([2, 1], "o_ps", "d", bufs=1)
    nc.tensor.matmul(o_ps[:, :], cb3_sb[0:40, C_FC2-C_HP:C_FC2-C_HP+2],
                     h[:, :])
    res = work.tile([2, 1], F32, name="res")
    nc.scalar.activation(res[:, :], o_ps[:, :], AF.Sigmoid,
                         bias=cf_sb[0:2, F_NFB2:F_NFB2+1], scale=1.0)

    nc.sync.dma_start(out=out_ap, in_=res[:, :])
    ctx.close()


_CACHE = {}


def build(debug_taps=False):
    key = ("nc", debug_taps)
    if key in _CACHE:
        return _CACHE[key]
    nc = bacc.Bacc("TRN2", target_bir_lowering=False, debug=False,
                   num_devices=N_CORES, num_swdge_queues=4,
                   dynamic_dma_scratch_size=65536)
    H = {name: nc.dram_tensor(name, list(shape), dt, kind="ExternalInput")
         for name, (shape, dt) in INPUT_SPECS.items()}
    out_t = nc.dram_tensor("out", [1, 2], F32, kind="ExternalOutput")
    with tile.TileContext(nc) as tc:
        _emit(nc, tc, H, out_t.ap())
    nc.compile()
    _CACHE[key] = nc
    return nc


def kernel(**inputs):
    nc = build()
    in_map = pack_inputs(inputs)
    res = run_bass_kernel_spmd(nc, [in_map] * N_CORES,
                               core_ids=list(range(N_CORES)))
    return res.results[0]["out"]


# revision 15
# speedup vs baseline: 1.1578x; 1.0395x over previous
"""Trainium2 Bass/Tile kernel for nn_CNN_77077483094746.

Single tiny sample (x: [1,1,18,140]) -> (1,2). The whole forward pass runs on
one NeuronCore; the same program is executed SPMD on all 8 cores (identical
inputs), output taken from core 0.

Strategy: everything that depends only on the WEIGHTS is precomputed on the
host in numpy (f64) and shipped as two packed constant tensors laid out
exactly as SBUF wants them:
  - cb (bf16, [128, XB]): all matmul operands — pre-transposed weights, the
    q/k projections folded into single Gram matrices G = Wq^T Wk (so
    S = [eeg,1] @ (Gpack @ kA^T + u) per branch, 2 matmuls instead of 3 and
    no weight transposes on device), block-diagonal packs for the four
    cross-modal branches (their q/k/v/out projections each become ONE
    matmul), the block-diagonal conv weights, fc weights.
  - cf (f32, [128, 128]): per-partition bias columns (bias folds: the value
    bias is folded into the output projection via softmax row-sums == 1).

The device program is ~50 LDWEIGHTS+MATMUL pairs, all bf16 (1 PE cycle/row),
5 input DMAs total. Only x-dependent compute runs on device. exp() is taken
without max-subtraction (|S| <~ 10 by construction, f32 range is safe), so
softmax is exp + accum rowsum + reciprocal + scale. Sigmoids use
1/(1+exp(-z)) on the already-loaded Exp table (no activation-table swap:
ReLU+bias and max-pool run on DVE).

Engine streams are emitted interleaved (A/B stage-1 chains, 4-way stage-2
branches) so PE never head-of-line blocks on DVE/ACT post-processing.
"""
import math
from contextlib import ExitStack

import numpy as np
import ml_dtypes

import concourse.bass as bass
import concourse.mybir as mybir
import concourse.tile as tile
from concourse import bacc
from concourse.bass_utils import run_bass_kernel_spmd
from concourse.masks import make_identity

WL = 140
OFC = 118
TDN = 21
D_CM = 16
N_BR = 4
C_OUT = 10
KS = 9
NCONV = OFC - KS + 1
F32 = mybir.dt.float32
BF16 = mybir.dt.bfloat16
N_CORES = 8

XB = 1048
XF = 256

# cb column layout (bf16 pack), three DMA chunks:
# chunk1 [128, 238]: GpackA [118,119] @0, GpackB @119
# chunk2 [128, 236]: M_A = WvT@owT [118,118] @0, M_B @118
# chunk3 [128, 574]: HPack 4x[16,17] @0 (rows at kv block base),
#   M2bd [80,64] @68, convwT [112, 9*40] @132, fc1T [40,40] @492,
#   fc2T [40,2] @532, projvec [1,32] @534, ones16 [16,1] @566
NB1 = 238
NB2 = 236
NB3 = 574
C_GPA = 0
C_GPB = 119
C_MA = 0
C_MB = 118
C_HP = 0
C_M2 = 68
C_CONV = 132
C_FC1 = 492
C_FC2 = 532
C_PROJ = 534
C_ONES = 566

# cf column layout (f32 pack)
F_UCA = 0        # ucol_A [119, 1]
F_UCB = 1        # ucol_B [119, 1]
F_OB16A = 2      # 16*obrow_A [118, 1]
F_OB16B = 3      # 16*obrow_B [118, 1]
F_OBROW_A = 4    # obrow_A at row 0: [1, 118]
F_OBROW_B = 128  # obrow_B at row 0: [1, 118]
F_U2 = 122       # per-branch u2col_i [17, 1] at F_U2+i (4 cols)
F_CONVB = 126    # convb [40, 1]
F_NFB1 = 127     # fc1_b [40, 1]
F_OB2 = 246      # per-branch ob2_i [16, 1] at F_OB2+i (4 cols)
F_NFB2 = 250     # fc2_b [2, 1]

INPUT_SPECS = {
    "x": ((1, 1, 18, WL), F32),
    "cb": ((128, XB), BF16),
    "cf": ((128, XF), F32),
}


def pack_consts(inp):
    """Host: all weight-only transforms, computed in f64."""
    s1 = 1.0 / math.sqrt(OFC)
    sb = 1.0 / math.sqrt(D_CM)
    cb = np.zeros((128, XB), np.float64)
    cf = np.zeros((128, XF), np.float64)
    cb1 = np.zeros((128, NB1), np.float64)
    cb2 = np.zeros((128, NB2), np.float64)
    cb3 = np.zeros((128, NB3), np.float64)
    for br, pre, cg, cm, fuc, fob16, obrow_c in (
            ("A", "tdA", C_GPA, C_MA, F_UCA, F_OB16A, F_OBROW_A),
            ("B", "tdB", C_GPB, C_MB, F_UCB, F_OB16B, F_OBROW_B)):
        in_w = np.asarray(inp[f"{pre}_in_w"], np.float64)
        in_b = np.asarray(inp[f"{pre}_in_b"], np.float64)
        out_w = np.asarray(inp[f"{pre}_out_w"], np.float64)
        out_b = np.asarray(inp[f"{pre}_out_b"], np.float64)
        wq, wk, wv = in_w[0:OFC], in_w[OFC:2*OFC], in_w[2*OFC:3*OFC]
        bq, bk, bv = in_b[0:OFC], in_b[OFC:2*OFC], in_b[2*OFC:3*OFC]
        cb1[0:OFC, cg:cg+OFC] = s1 * (wq.T @ wk).T         # Gpack[j, i]=s1*G[i,j]
        cb1[0:OFC, cg+OFC] = s1 * (wk.T @ bq)              # v-row
        cb2[0:OFC, cm:cm+OFC] = wv.T @ out_w.T             # M: att = P^T (kA M)
        cf[0:OFC, fuc] = s1 * (wq.T @ bk)
        cf[OFC, fuc] = s1 * (bq @ bk)
        obr = out_w @ bv + out_b
        cf[0:OFC, fob16] = 16.0 * obr
        cf[0, obrow_c:obrow_c+OFC] = obr
    cb3[0, C_PROJ:C_PROJ+16] = np.asarray(inp["projA_w"], np.float64)[:, 0]
    cb3[0, C_PROJ+16:C_PROJ+32] = np.asarray(inp["projB_w"], np.float64)[:, 0]
    cb3[0:16, C_ONES] = 1.0

    cm_in_w = np.asarray(inp["cm_in_w"], np.float64)
    cm_in_b = np.asarray(inp["cm_in_b"], np.float64)
    cm_out_w = np.asarray(inp["cm_out_w"], np.float64)
    cm_out_b = np.asarray(inp["cm_out_b"], np.float64)
    # KV row-block base per branch (kv = [eeg, wA, wB, eeg]; blocks at
    # partition bases 0/32/64, branch 3 reuses the eeg block at 0)
    kvb = [0, 32, 64, 0]
    for i in range(N_BR):
        wq, wk, wv = (cm_in_w[i, 0:D_CM], cm_in_w[i, D_CM:2*D_CM],
                      cm_in_w[i, 2*D_CM:3*D_CM])
        bq, bk, bv = (cm_in_b[i, 0:D_CM], cm_in_b[i, D_CM:2*D_CM],
                      cm_in_b[i, 2*D_CM:3*D_CM])
        cb3[kvb[i]:kvb[i]+16, C_HP+17*i:C_HP+17*i+16] = sb * (wq.T @ wk).T
        cb3[kvb[i]:kvb[i]+16, C_HP+17*i+16] = sb * (wk.T @ bq)
        cf[0:16, F_U2+i] = sb * (wq.T @ bk)
        cf[16, F_U2+i] = sb * (bq @ bk)
        # M2 block: oT_i = attv2_i^T @ P2n_i^T with attv2 = kv @ (WvT WoT)
        cb3[kvb[i]:kvb[i]+16, C_M2+16*i:C_M2+16*i+16] = wv.T @ cm_out_w[i].T
        cf[0:16, F_OB2+i] = cm_out_w[i] @ bv + cm_out_b[i]
    cw = np.asarray(inp["conv_w"], np.float64)
    for i in range(N_BR):
        # convwT[32i+c, k, 10i+oc] = conv_w[i, oc, c, k] (oTall blocks at 32i)
        for k in range(KS):
            cb3[32*i:32*i+16, C_CONV+40*k+10*i:C_CONV+40*k+10*i+10] = cw[i, :, :, k].T
    cf[0:40, F_CONVB] = np.asarray(inp["conv_b"], np.float64).reshape(40)
    cb3[0:40, C_FC1:C_FC1+40] = np.asarray(inp["fc1_w"], np.float64).T
    cf[0:40, F_NFB1] = np.asarray(inp["fc1_b"], np.float64)
    cb3[0:40, C_FC2:C_FC2+2] = np.asarray(inp["fc2_w"], np.float64).T
    cf[0:2, F_NFB2] = np.asarray(inp["fc2_b"], np.float64)
    cb = np.concatenate([cb1, cb2, cb3], axis=1)
    assert cb.shape[1] == XB, cb.shape
    return (np.ascontiguousarray(cb.astype(ml_dtypes.bfloat16)),
            np.ascontiguousarray(cf.astype(np.float32)))


def pack_inputs(inputs):
    cb, cf = pack_consts(inputs)
    x = np.ascontiguousarray(np.asarray(inputs["x"]), dtype=np.float32)
    return {"x": x, "cb": cb, "cf": cf}


def _emit(nc, tc, H, out_ap):
    AF = mybir.ActivationFunctionType
    ALU = mybir.AluOpType
    X = mybir.AxisListType.X

    ctx = ExitStack()
    consts = ctx.enter_context(tc.tile_pool(name="consts", bufs=1))
    work = ctx.enter_context(tc.tile_pool(name="work", bufs=1))
    psum = ctx.enter_context(tc.tile_pool(name="psum", bufs=1, space="PSUM"))

    def dram_ap(handle, off, dims):
        return bass.AP(tensor=handle, offset=off, ap=[list(d) for d in dims])

    def pst(shape, nm, tag, bufs=2, dt=F32):
        return psum.tile(shape, dt, name=nm, tag=tag, bufs=bufs)

    x_h = H["x"]

    # ============================ DMA issue ===============================
    # Few fragments per DMA: per-DMA latency is ~2.2us + ~6.5ns/fragment.
    cb1_sb = consts.tile([128, NB1], BF16, name="cb1")
    cb2_sb = consts.tile([128, NB2], BF16, name="cb2")
    cb3_sb = consts.tile([128, NB3], BF16, name="cb3")
    cf_sb = consts.tile([128, XF], F32, name="cf")
    kab_nat = work.tile([TDN, 2, OFC], F32, name="kab_nat")
    eeg_raw = work.tile([16, OFC], F32, name="eeg_raw")

    # gpsimd SWDGE: kab (42 contiguous 472B fragments; arrives first),
    # then the f32 bias pack (first consumed late)
    nc.gpsimd.dma_start(out=kab_nat[:, :, :],
                        in_=dram_ap(x_h, 0, [(1, TDN), (17 * WL, 2), (1, OFC)]))
    nc.gpsimd.dma_start(out=cf_sb[:, :], in_=dram_ap(H["cf"], 0, [(XF, 128), (1, XF)]))
    # SP: Gpack chunk first (gates the first matmul), then WvT/owT
    nc.sync.dma_start(out=cb1_sb[:, :], in_=dram_ap(H["cb"], 0, [(XB, 128), (1, NB1)]))
    nc.sync.dma_start(out=cb2_sb[:, :],
                      in_=dram_ap(H["cb"], NB1, [(XB, 128), (1, NB2)]))
    # ACT: eeg rows first, then the late-phase const chunk
    nc.scalar.dma_start(out=eeg_raw[:, :],
                        in_=dram_ap(x_h, WL + (WL - OFC), [(WL, 16), (1, OFC)]))
    nc.scalar.dma_start(out=cb3_sb[:, :],
                        in_=dram_ap(H["cb"], NB1 + NB2, [(XB, 128), (1, NB3)]))

    # ======================= early prep (gpsimd/DVE) ======================
    identity = consts.tile([128, 128], BF16, name="identity")
    make_identity(nc, identity)

    eeg_ext = work.tile([16, OFC + 1], BF16, name="eeg_ext")
    nc.gpsimd.memset(eeg_ext[:, OFC:OFC+1], 1.0)

    dataA = work.tile([17, OFC], BF16, name="dataA")
    dataE = work.tile([17, OFC], BF16, name="dataE")
    dataB = work.tile([17, OFC], BF16, name="dataB")
    KV = work.tile([80, OFC], BF16, name="KV")
    oTall = work.tile([112, OFC], BF16, name="oTall")
    for t in (dataA, dataE, dataB):
        nc.gpsimd.memset(t[:, :], 1.0)  # row 16 stays ones; 0:16 overwritten
    nc.gpsimd.memset(KV[:, :], 0.0)     # gap rows must be 0 for the packs
    nc.gpsimd.memset(oTall[:, :], 0.0)  # gap rows hit nonzero conv weights? no:
                                        # conv weights are 0 there; avoid NaNs

    kabn_b = work.tile([TDN, 2, OFC], BF16, name="kabn_b")
    nc.vector.tensor_copy(kabn_b[:, :, :], kab_nat[:, :, :])
    kabT_ps = pst([OFC, 2, TDN + 1], "kabT_ps", "s2", bufs=1, dt=BF16)
    nc.tensor.transpose(kabT_ps[:, 0, 0:TDN], kabn_b[:, 0, :], identity[0:TDN, 0:TDN])
    nc.tensor.transpose(kabT_ps[:, 1, 0:TDN], kabn_b[:, 1, :], identity[0:TDN, 0:TDN])
    kab_b = work.tile([OFC, 2, TDN], BF16, name="kab_b")
    nc.vector.tensor_copy(kab_b[:, 0, :], kabT_ps[:, 0, 0:TDN])
    nc.vector.tensor_copy(kab_b[:, 1, :], kabT_ps[:, 1, 0:TDN])
    nc.scalar.copy(eeg_ext[:, 0:OFC], eeg_raw[:, :])
    # stage-2 eeg rows (off critical path; Pool engine)
    nc.gpsimd.tensor_copy(KV[0:16, :], eeg_raw[:, :])
    nc.gpsimd.tensor_copy(dataE[0:16, :], eeg_raw[:, :])

    # eeg^T (with ones row 118) via PE transpose
    eegT_ps = pst([OFC + 1, 16], "eegT_ps", "c", dt=BF16)
    nc.tensor.transpose(eegT_ps[:, :], eeg_ext[:, :], identity[0:16, 0:16])
    eegT = work.tile([OFC + 1, 16], BF16, name="eegT")
    nc.vector.tensor_copy(eegT[:, :], eegT_ps[:, :])

    # ===================== stage 1 (A/B interleaved) ======================
    GP = {"A": cb1_sb[0:OFC, C_GPA:C_GPA+OFC+1],
          "B": cb1_sb[0:OFC, C_GPB:C_GPB+OFC+1]}
    MM = {"A": cb2_sb[0:OFC, C_MA:C_MA+OFC], "B": cb2_sb[0:OFC, C_MB:C_MB+OFC]}
    UC = {"A": cf_sb[0:OFC+1, F_UCA:F_UCA+1], "B": cf_sb[0:OFC+1, F_UCB:F_UCB+1]}
    OB16 = {"A": cf_sb[0:OFC, F_OB16A:F_OB16A+1], "B": cf_sb[0:OFC, F_OB16B:F_OB16B+1]}
    OBROW = {"A": cf_sb[0:1, F_OBROW_A:F_OBROW_A+OFC],
             "B": cf_sb[0:1, F_OBROW_B:F_OBROW_B+OFC]}
    PROJ = {"A": cb3_sb[0:1, C_PROJ:C_PROJ+16],
            "B": cb3_sb[0:1, C_PROJ+16:C_PROJ+32]}
    kT = {"A": kab_b[:, 0, :], "B": kab_b[:, 1, :]}
    tag1 = {"A": "a", "B": "b"}
    cpe = {"A": nc.vector, "B": nc.scalar}  # PSUM->SBUF copy engine per branch

    def cp(eng, out, in_):
        (eng.tensor_copy if eng is nc.vector else eng.copy)(out, in_)

    def cpadd(eng, out, in_, bias):
        if eng is nc.vector:
            eng.tensor_scalar_add(out, in_, bias)
        else:
            eng.add(out, in_, bias)

    s1 = {"A": {}, "B": {}}

    def ps1(br, shape, nm):
        return pst(shape, f"{nm}_{br}", tag1[br])

    def gk_mm(br):
        d = s1[br]
        d["gk_ps"] = ps1(br, [OFC + 1, TDN], "gk")
        nc.tensor.matmul(d["gk_ps"][:, :], GP[br], kT[br])

    def gk_post(br):
        d = s1[br]
        d["gk"] = work.tile([OFC + 1, TDN], BF16, name=f"gk_{br}")
        cpadd(cpe[br], d["gk"][:, :], d["gk_ps"][:, :], UC[br])

    def vp_mm(br):
        d = s1[br]
        d["vp_ps"] = ps1(br, [TDN, OFC], "vp")
        nc.tensor.matmul(d["vp_ps"][:, :], kT[br], MM[br])

    def vp_post(br):
        d = s1[br]
        d["vp"] = work.tile([TDN, OFC], BF16, name=f"vp_{br}")
        cp(cpe[br], d["vp"][:, :], d["vp_ps"][:, :])

    def s_mm(br):
        d = s1[br]
        d["S_ps"] = ps1(br, [16, TDN], "S")
        nc.tensor.matmul(d["S_ps"][:, :], eegT[:, :], d["gk"][:, :])

    def softmax1(br):
        d = s1[br]
        d["P"] = work.tile([16, TDN], BF16, name=f"P_{br}")
        nc.scalar.activation(d["P"][:, :], d["S_ps"][:, :], AF.Exp)

    def rinv1(br):
        # off the critical path: attnT/ZT/att use unnormalized P; the
        # normalization lands on the att PSUM->SBUF copy (per-q scale)
        d = s1[br]
        d["rowsum"] = work.tile([16, 1], F32, name=f"rowsum_{br}")
        nc.vector.reduce_sum(d["rowsum"][:, :], d["P"][:, :], axis=X)
        d["rinv"] = work.tile([16, 1], F32, name=f"rinv_{br}")
        nc.vector.reciprocal(d["rinv"][:, :], d["rowsum"][:, :])

    def attnT_t(br):
        d = s1[br]
        d["aT_ps"] = pst([TDN, 16], f"aT_{br}", tag1[br], dt=BF16)
        nc.tensor.transpose(d["aT_ps"][:, :], d["P"][:, :], identity[0:16, 0:16])

    def attnT_cp(br):
        d = s1[br]
        d["aT"] = work.tile([TDN, 16], BF16, name=f"aT_{br}")
        cp(cpe[br], d["aT"][:, :], d["aT_ps"][:, :])

    def att_mm(br):
        d = s1[br]
        d["att_ps"] = ps1(br, [16, OFC], "att")
        nc.tensor.matmul(d["att_ps"][:, :], d["aT"][:, :], d["vp"][:, :])

    def att_cp(br):
        d = s1[br]
        d["att"] = work.tile([16, OFC], BF16, name=f"att_{br}")
        if br == "A":
            nc.vector.tensor_scalar_mul(d["att"][:, :], d["att_ps"][:, :],
                                        d["rinv"][:, :])
        else:
            nc.scalar.activation(d["att"][:, :], d["att_ps"][:, :], AF.Copy,
                                 scale=d["rinv"][:, :])

    def svec_mm(br):
        d = s1[br]
        d["svec_ps"] = ps1(br, [OFC, 1], "svec")
        nc.tensor.matmul(d["svec_ps"][:, :], d["att"][:, :],
                         cb3_sb[0:16, C_ONES:C_ONES+1])

    def svec_post(br):
        d = s1[br]
        d["svec"] = work.tile([OFC, 1], BF16, name=f"svec_{br}")
        cpadd(cpe[br], d["svec"][:, :], d["svec_ps"][:, :], OB16[br])

    def sc_mm(br):
        d = s1[br]
        d["sc_ps"] = ps1(br, [1, 16], "sc")
        nc.tensor.matmul(d["sc_ps"][:, :], d["svec"][:, :], eegT[0:OFC, :])

    def sel_post(br):
        d = s1[br]
        d["m"] = work.tile([1, 1], F32, name=f"m_{br}")
        nc.vector.reduce_max(d["m"][:, :], d["sc_ps"][:, :], axis=X)
        d["ohr"] = work.tile([1, 16], BF16, name=f"ohr_{br}")
        nc.vector.tensor_scalar(d["ohr"][:, :], d["sc_ps"][:, :], d["m"][:, :],
                                None, op0=ALU.is_equal)

    def oh_t(br):
        d = s1[br]
        d["oh_ps"] = pst([16, 1], f"oh_{br}", tag1[br], dt=BF16)
        nc.tensor.transpose(d["oh_ps"][:, :], d["ohr"][:, :], identity[0:1, 0:1])

    def oh_cp(br):
        d = s1[br]
        d["oh"] = work.tile([16, 1], BF16, name=f"oh_{br}")
        cp(cpe[br], d["oh"][:, :], d["oh_ps"][:, :])

    def row_mm(br):
        d = s1[br]
        d["row_ps"] = ps1(br, [1, OFC], "row")
        nc.tensor.matmul(d["row_ps"][:, :], d["oh"][:, :], d["att"][:, :])

    def row_post(br):
        d = s1[br]
        d["row"] = work.tile([1, OFC], BF16, name=f"row_{br}")
        nc.vector.tensor_add(d["row"][:, :], d["row_ps"][:, :], OBROW[br])

    def w_mm(br):
        d = s1[br]
        d["w_ps"] = ps1(br, [16, OFC], "w")
        nc.tensor.matmul(d["w_ps"][:, :], PROJ[br], d["row"][:, :])

    def w_cp(br):
        # wA -> DATA rows 0:16 and KV rows 16:32; wB -> DATA 51:67, KV 32:48
        d = s1[br]
        if br == "A":
            nc.vector.tensor_copy(dataA[0:16, :], d["w_ps"][:, :])
            nc.scalar.copy(KV[32:48, :], d["w_ps"][:, :])
        else:
            nc.vector.tensor_copy(dataB[0:16, :], d["w_ps"][:, :])
            nc.scalar.copy(KV[64:80, :], d["w_ps"][:, :])

    gk_mm("A")
    gk_mm("B")
    gk_post("A")
    vp_mm("A")
    gk_post("B")
    vp_mm("B")
    s_mm("A")
    vp_post("A")
    s_mm("B")
    softmax1("A")
    vp_post("B")
    rinv1("A")
    softmax1("B")
    attnT_t("A")
    attnT_cp("A")
    rinv1("B")
    att_mm("A")
    attnT_t("B")
    att_cp("A")
    attnT_cp("B")
    svec_mm("A")
    att_mm("B")
    svec_post("A")
    att_cp("B")
    sc_mm("A")
    svec_mm("B")
    sel_post("A")
    svec_post("B")
    oh_t("A")
    sc_mm("B")
    oh_cp("A")
    sel_post("B")
    row_mm("A")
    oh_t("B")
    row_post("A")
    oh_cp("B")
    w_mm("A")
    row_mm("B")
    w_cp("A")
    row_post("B")
    w_mm("B")
    w_cp("B")

    # ===================== stage 2 (4-way lockstep) =======================
    cpe2 = [nc.vector, nc.scalar, nc.vector, nc.scalar]
    kvb = [0, 32, 64, 0]
    hx_ps = [pst([D_CM + 1, OFC], f"hx_ps_{i}", "c" if i % 2 == 0 else "d",
                 bufs=2 if i % 2 == 0 else 1) for i in range(N_BR)]
    for i in range(N_BR):
        nc.tensor.matmul(hx_ps[i][:, :],
                         cb3_sb[kvb[i]:kvb[i]+16, C_HP+17*i:C_HP+17*i+17],
                         KV[kvb[i]:kvb[i]+16, :])
    vp2_ps = pst([OFC, 64], "vp2_ps", "d", bufs=1)
    nc.tensor.matmul(vp2_ps[:, :], KV[:, :], cb3_sb[0:80, C_M2:C_M2+64])
    hx = [work.tile([D_CM + 1, OFC], BF16, name=f"hx_{i}") for i in range(N_BR)]
    for i in range(N_BR):
        cpadd(cpe2[i], hx[i][:, :], hx_ps[i][:, :], cf_sb[0:17, F_U2+i:F_U2+i+1])
    vp2 = work.tile([OFC, 64], BF16, name="vp2")
    nc.scalar.copy(vp2[:, :], vp2_ps[:, :])

    data2 = [dataA, dataE, dataE, dataB]
    S2_ps = pst([OFC, N_BR * OFC], "S2_ps", "s2", bufs=1)
    for i in range(N_BR):
        nc.tensor.matmul(S2_ps[:, OFC*i:OFC*(i+1)], data2[i][:, :], hx[i][:, :])

    b = [dict() for _ in range(N_BR)]
    for i in range(N_BR):
        b[i]["P"] = work.tile([OFC, OFC], BF16, name=f"P2_{i}")
        nc.scalar.activation(b[i]["P"][:, :], S2_ps[:, OFC*i:OFC*(i+1)], AF.Exp)
        b[i]["rs"] = work.tile([OFC, 1], F32, name=f"rs2_{i}")
        nc.vector.reduce_sum(b[i]["rs"][:, :], b[i]["P"][:, :], axis=X)
        b[i]["rinv"] = work.tile([OFC, 1], F32, name=f"rinv2_{i}")
        nc.vector.reciprocal(b[i]["rinv"][:, :], b[i]["rs"][:, :])
        b[i]["Pn"] = work.tile([OFC, OFC], BF16, name=f"Pn2_{i}")
        if i % 2 == 0:
            nc.vector.tensor_scalar_mul(b[i]["Pn"][:, :], b[i]["P"][:, :],
                                        b[i]["rinv"][:, :])
        else:
            nc.scalar.activation(b[i]["Pn"][:, :], b[i]["P"][:, :], AF.Copy,
                                 scale=b[i]["rinv"][:, :])
        b[i]["aT_ps"] = pst([OFC, OFC], f"aT2_{i}", "a" if i % 2 == 0 else "b",
                            dt=BF16)
        nc.tensor.transpose(b[i]["aT_ps"][:, :], b[i]["Pn"][:, :],
                            identity[0:OFC, 0:OFC])
        b[i]["aT"] = work.tile([OFC, OFC], BF16, name=f"aT2_{i}")
        cp(cpe2[i], b[i]["aT"][:, :], b[i]["aT_ps"][:, :])

    for i in range(N_BR):
        b[i]["oT_ps"] = pst([D_CM, OFC], f"oT2_{i}", "c" if i % 2 == 0 else "d",
                            bufs=2 if i % 2 == 0 else 1)
        nc.tensor.matmul(b[i]["oT_ps"][:, :], vp2[:, 16*i:16*(i+1)],
                         b[i]["aT"][:, :])
        cpadd(cpe2[i], oTall[32*i:32*i+16, :], b[i]["oT_ps"][:, :],
              cf_sb[0:16, F_OB2+i:F_OB2+i+1])

    # ======================== conv + classifier ===========================
    y_ps = pst([4 * C_OUT, NCONV], "y_ps", "d", bufs=1)
    for k in range(KS):
        nc.tensor.matmul(y_ps[:, :],
                         cb3_sb[0:112, C_CONV+40*k:C_CONV+40*k+40],
                         oTall[0:112, k:k+NCONV], start=(k == 0),
                         stop=(k == KS - 1))
    relu = work.tile([4 * C_OUT, NCONV], F32, name="relu")
    nc.vector.tensor_scalar(relu[:, :], y_ps[:, :], cf_sb[0:40, F_CONVB:F_CONVB+1],
                            0.0, op0=ALU.add, op1=ALU.max)
    feat = work.tile([4 * C_OUT, 1], BF16, name="feat")
    nc.vector.reduce_max(feat[:, :], relu[:, :], axis=X)

    h_ps = pst([40, 1], "h_ps", "a")
    nc.tensor.matmul(h_ps[:, :], cb3_sb[0:40, C_FC1:C_FC1+40], feat[:, :])
    h = work.tile([40, 1], BF16, name="h")
    # bias holds +fc1_b here (negfb1 slot repurposed on host as +fb)
    nc.scalar.activation(h[:, :], h_ps[:, :], AF.Sigmoid,
                         bias=cf_sb[0:40, F_NFB1:F_NFB1+1], scale=1.0)

    o_ps = pst([2, 1], "o_ps", "d", bufs=1)
    nc.tensor.matmul(o_ps[:, :], cb3_sb[0:40, C_FC2:C_FC2+2], h[:, :])
    res = work.tile([2, 1], F32, name="res")
    nc.scalar.activation(res[:, :], o_ps[:, :], AF.Sigmoid,
                         bias=cf_sb[0:2, F_NFB2:F_NFB2+1], scale=1.0)

    nc.sync.dma_start(out=out_ap, in_=res[:, :])
    ctx.close()


_CACHE = {}


def build(debug_taps=False):
    key = ("nc", debug_taps)
    if key in _CACHE:
        return _CACHE[key]
    nc = bacc.Bacc("TRN2", target_bir_lowering=False, debug=False,
                   num_devices=N_CORES, num_swdge_queues=4,
                   dynamic_dma_scratch_size=65536)
    H = {name: nc.dram_tensor(name, list(shape), dt, kind="ExternalInput")
         for name, (shape, dt) in INPUT_SPECS.items()}
    out_t = nc.dram_tensor("out", [1, 2], F32, kind="ExternalOutput")
    with tile.TileContext(nc) as tc:
        _emit(nc, tc, H, out_t.ap())
    nc.compile()
    _CACHE[key] = nc
    return nc


def kernel(**inputs):
    nc = build()
    in_map = pack_inputs(inputs)
    res = run_bass_kernel_spmd(nc, [in_map] * N_CORES,
                               core_ids=list(range(N_CORES)))
    return res.results[0]["out"]


# revision 17
# speedup vs baseline: 1.2064x; 1.0420x over previous
"""Trainium2 Bass/Tile kernel for nn_CNN_77077483094746.

Single tiny sample (x: [1,1,18,140]) -> (1,2). The whole forward pass runs on
one NeuronCore; the same program is executed SPMD on all 8 cores (identical
inputs), output taken from core 0.

Strategy: everything that depends only on the WEIGHTS is precomputed on the
host in numpy (f64) and shipped as two packed constant tensors laid out
exactly as SBUF wants them:
  - cb (bf16, [128, XB]): all matmul operands — pre-transposed weights, the
    q/k projections folded into single Gram matrices G = Wq^T Wk (so
    S = [eeg,1] @ (Gpack @ kA^T + u) per branch, 2 matmuls instead of 3 and
    no weight transposes on device), block-diagonal packs for the four
    cross-modal branches (their q/k/v/out projections each become ONE
    matmul), the block-diagonal conv weights, fc weights.
  - cf (f32, [128, 128]): per-partition bias columns (bias folds: the value
    bias is folded into the output projection via softmax row-sums == 1).

The device program is ~50 LDWEIGHTS+MATMUL pairs, all bf16 (1 PE cycle/row),
5 input DMAs total. Only x-dependent compute runs on device. exp() is taken
without max-subtraction (|S| <~ 10 by construction, f32 range is safe), so
softmax is exp + accum rowsum + reciprocal + scale. Sigmoids use
1/(1+exp(-z)) on the already-loaded Exp table (no activation-table swap:
ReLU+bias and max-pool run on DVE).

Engine streams are emitted interleaved (A/B stage-1 chains, 4-way stage-2
branches) so PE never head-of-line blocks on DVE/ACT post-processing.
"""
import math
from contextlib import ExitStack

import numpy as np
import ml_dtypes

import concourse.bass as bass
import concourse.mybir as mybir
import concourse.tile as tile
from concourse import bacc
from concourse.bass_utils import run_bass_kernel_spmd
from concourse.masks import make_identity

WL = 140
OFC = 118
TDN = 21
D_CM = 16
N_BR = 4
C_OUT = 10
KS = 9
NCONV = OFC - KS + 1
F32 = mybir.dt.float32
BF16 = mybir.dt.bfloat16
N_CORES = 8

XB = 1048
XF = 256

# cb column layout (bf16 pack), three DMA chunks:
# chunk1 [128, 238]: GpackA [118,119] @0, GpackB @119
# chunk2 [128, 236]: M_A = WvT@owT [118,118] @0, M_B @118
# chunk3 [128, 574]: HPack 4x[16,17] @0 (rows at kv block base),
#   M2bd [80,64] @68, convwT [112, 9*40] @132, fc1T [40,40] @492,
#   fc2T [40,2] @532, projvec [1,32] @534, ones16 [16,1] @566
NB1 = 238
NB2 = 236
NB3 = 574
C_GPA = 0
C_GPB = 119
C_MA = 0
C_MB = 118
C_HP = 0
C_M2 = 68
C_CONV = 132
C_FC1 = 492
C_FC2 = 532
C_PROJ = 534
C_ONES = 566

# cf column layout (f32 pack)
F_UCA = 0        # ucol_A [119, 1]
F_UCB = 1        # ucol_B [119, 1]
F_OB16A = 2      # 16*obrow_A [118, 1]
F_OB16B = 3      # 16*obrow_B [118, 1]
F_OBROW_A = 4    # obrow_A at row 0: [1, 118]
F_OBROW_B = 128  # obrow_B at row 0: [1, 118]
F_U2 = 122       # per-branch u2col_i [17, 1] at F_U2+i (4 cols)
F_CONVB = 126    # convb [40, 1]
F_NFB1 = 127     # fc1_b [40, 1]
F_OB2 = 246      # per-branch ob2_i [16, 1] at F_OB2+i (4 cols)
F_NFB2 = 250     # fc2_b [2, 1]

INPUT_SPECS = {
    "x": ((1, 1, 18, WL), F32),
    "cb": ((128, XB), BF16),
    "cf": ((128, XF), F32),
}


def pack_consts(inp):
    """Host: all weight-only transforms, computed in f64."""
    s1 = 1.0 / math.sqrt(OFC)
    sb = 1.0 / math.sqrt(D_CM)
    cb = np.zeros((128, XB), np.float64)
    cf = np.zeros((128, XF), np.float64)
    cb1 = np.zeros((128, NB1), np.float64)
    cb2 = np.zeros((128, NB2), np.float64)
    cb3 = np.zeros((128, NB3), np.float64)
    for br, pre, cg, cm, fuc, fob16, obrow_c in (
            ("A", "tdA", C_GPA, C_MA, F_UCA, F_OB16A, F_OBROW_A),
            ("B", "tdB", C_GPB, C_MB, F_UCB, F_OB16B, F_OBROW_B)):
        in_w = np.asarray(inp[f"{pre}_in_w"], np.float64)
        in_b = np.asarray(inp[f"{pre}_in_b"], np.float64)
        out_w = np.asarray(inp[f"{pre}_out_w"], np.float64)
        out_b = np.asarray(inp[f"{pre}_out_b"], np.float64)
        wq, wk, wv = in_w[0:OFC], in_w[OFC:2*OFC], in_w[2*OFC:3*OFC]
        bq, bk, bv = in_b[0:OFC], in_b[OFC:2*OFC], in_b[2*OFC:3*OFC]
        cb1[0:OFC, cg:cg+OFC] = s1 * (wq.T @ wk).T         # Gpack[j, i]=s1*G[i,j]
        cb1[0:OFC, cg+OFC] = s1 * (wk.T @ bq)              # v-row
        cb2[0:OFC, cm:cm+OFC] = wv.T @ out_w.T             # M: att = P^T (kA M)
        cf[0:OFC, fuc] = s1 * (wq.T @ bk)
        cf[OFC, fuc] = s1 * (bq @ bk)
        obr = out_w @ bv + out_b
        cf[0:OFC, fob16] = 16.0 * obr
        cf[0, obrow_c:obrow_c+OFC] = obr
    cb3[0, C_PROJ:C_PROJ+16] = np.asarray(inp["projA_w"], np.float64)[:, 0]
    cb3[0, C_PROJ+16:C_PROJ+32] = np.asarray(inp["projB_w"], np.float64)[:, 0]
    cb3[0:16, C_ONES] = 1.0

    cm_in_w = np.asarray(inp["cm_in_w"], np.float64)
    cm_in_b = np.asarray(inp["cm_in_b"], np.float64)
    cm_out_w = np.asarray(inp["cm_out_w"], np.float64)
    cm_out_b = np.asarray(inp["cm_out_b"], np.float64)
    # KV row-block base per branch (kv = [eeg, wA, wB, eeg]; blocks at
    # partition bases 0/32/64, branch 3 reuses the eeg block at 0)
    kvb = [0, 32, 64, 0]
    for i in range(N_BR):
        wq, wk, wv = (cm_in_w[i, 0:D_CM], cm_in_w[i, D_CM:2*D_CM],
                      cm_in_w[i, 2*D_CM:3*D_CM])
        bq, bk, bv = (cm_in_b[i, 0:D_CM], cm_in_b[i, D_CM:2*D_CM],
                      cm_in_b[i, 2*D_CM:3*D_CM])
        cb3[kvb[i]:kvb[i]+16, C_HP+17*i:C_HP+17*i+16] = sb * (wq.T @ wk).T
        cb3[kvb[i]:kvb[i]+16, C_HP+17*i+16] = sb * (wk.T @ bq)
        cf[0:16, F_U2+i] = sb * (wq.T @ bk)
        cf[16, F_U2+i] = sb * (bq @ bk)
        # M2 block: oT_i = attv2_i^T @ P2n_i^T with attv2 = kv @ (WvT WoT)
        cb3[kvb[i]:kvb[i]+16, C_M2+16*i:C_M2+16*i+16] = wv.T @ cm_out_w[i].T
        cf[0:16, F_OB2+i] = cm_out_w[i] @ bv + cm_out_b[i]
    cw = np.asarray(inp["conv_w"], np.float64)
    for i in range(N_BR):
        # convwT[32i+c, k, 10i+oc] = conv_w[i, oc, c, k] (oTall blocks at 32i)
        for k in range(KS):
            cb3[32*i:32*i+16, C_CONV+40*k+10*i:C_CONV+40*k+10*i+10] = cw[i, :, :, k].T
    cf[0:40, F_CONVB] = np.asarray(inp["conv_b"], np.float64).reshape(40)
    cb3[0:40, C_FC1:C_FC1+40] = np.asarray(inp["fc1_w"], np.float64).T
    cf[0:40, F_NFB1] = np.asarray(inp["fc1_b"], np.float64)
    cb3[0:40, C_FC2:C_FC2+2] = np.asarray(inp["fc2_w"], np.float64).T
    cf[0:2, F_NFB2] = np.asarray(inp["fc2_b"], np.float64)
    cb = np.concatenate([cb1, cb2, cb3], axis=1)
    assert cb.shape[1] == XB, cb.shape
    return (np.ascontiguousarray(cb.astype(ml_dtypes.bfloat16)),
            np.ascontiguousarray(cf.astype(np.float32)))


def pack_inputs(inputs):
    cb, cf = pack_consts(inputs)
    x = np.ascontiguousarray(np.asarray(inputs["x"]), dtype=np.float32)
    return {"x": x, "cb": cb, "cf": cf}


def _emit(nc, tc, H, out_ap):
    AF = mybir.ActivationFunctionType
    ALU = mybir.AluOpType
    X = mybir.AxisListType.X

    ctx = ExitStack()
    consts = ctx.enter_context(tc.tile_pool(name="consts", bufs=1))
    work = ctx.enter_context(tc.tile_pool(name="work", bufs=1))
    psum = ctx.enter_context(tc.tile_pool(name="psum", bufs=1, space="PSUM"))

    def dram_ap(handle, off, dims):
        return bass.AP(tensor=handle, offset=off, ap=[list(d) for d in dims])

    def pst(shape, nm, tag, bufs=2, dt=F32):
        return psum.tile(shape, dt, name=nm, tag=tag, bufs=bufs)

    x_h = H["x"]

    # ============================ DMA issue ===============================
    # Few fragments per DMA: per-DMA latency is ~2.2us + ~6.5ns/fragment.
    cb1_sb = consts.tile([128, NB1], BF16, name="cb1")
    cb2_sb = consts.tile([128, NB2], BF16, name="cb2")
    cb3_sb = consts.tile([128, NB3], BF16, name="cb3")
    cf_sb = consts.tile([128, XF], F32, name="cf")
    kab_nat = work.tile([TDN, 2, OFC], F32, name="kab_nat")
    eeg_raw = work.tile([16, OFC], F32, name="eeg_raw")

    # gpsimd SWDGE: kab (42 contiguous 472B fragments; arrives first),
    # then the f32 bias pack (first consumed late)
    nc.gpsimd.dma_start(out=kab_nat[:, :, :],
                        in_=dram_ap(x_h, 0, [(1, TDN), (17 * WL, 2), (1, OFC)]))
    nc.gpsimd.dma_start(out=cf_sb[:, :], in_=dram_ap(H["cf"], 0, [(XF, 128), (1, XF)]))
    # SP: Gpack chunk first (gates the first matmul), then WvT/owT
    nc.sync.dma_start(out=cb1_sb[:, :], in_=dram_ap(H["cb"], 0, [(XB, 128), (1, NB1)]))
    nc.sync.dma_start(out=cb2_sb[:, :],
                      in_=dram_ap(H["cb"], NB1, [(XB, 128), (1, NB2)]))
    # ACT: eeg rows first, then the late-phase const chunk
    nc.scalar.dma_start(out=eeg_raw[:, :],
                        in_=dram_ap(x_h, WL + (WL - OFC), [(WL, 16), (1, OFC)]))
    nc.scalar.dma_start(out=cb3_sb[:, :],
                        in_=dram_ap(H["cb"], NB1 + NB2, [(XB, 128), (1, NB3)]))

    # ======================= early prep (gpsimd/DVE) ======================
    identity = consts.tile([128, 128], BF16, name="identity")
    make_identity(nc, identity)
    idf32 = consts.tile([32, 32], F32, name="idf32")
    make_identity(nc, idf32)

    eeg_ext = work.tile([16, OFC + 1], BF16, name="eeg_ext")
    nc.gpsimd.memset(eeg_ext[:, OFC:OFC+1], 1.0)

    dataA = work.tile([17, OFC], BF16, name="dataA")
    dataE = work.tile([17, OFC], BF16, name="dataE")
    dataB = work.tile([17, OFC], BF16, name="dataB")
    KV = work.tile([80, OFC], BF16, name="KV")
    oTall = work.tile([112, OFC], BF16, name="oTall")
    for t in (dataA, dataE, dataB):
        nc.gpsimd.memset(t[:, :], 1.0)  # row 16 stays ones; 0:16 overwritten
    nc.gpsimd.memset(KV[:, :], 0.0)     # gap rows must be 0 for the packs
    nc.gpsimd.memset(oTall[:, :], 0.0)  # gap rows hit nonzero conv weights? no:
                                        # conv weights are 0 there; avoid NaNs

    kabT_ps = pst([OFC, 2, TDN + 1], "kabT_ps", "s2", bufs=1)
    nc.tensor.transpose(kabT_ps[:, 0, 0:TDN], kab_nat[:, 0, :], idf32[0:TDN, 0:TDN])
    nc.tensor.transpose(kabT_ps[:, 1, 0:TDN], kab_nat[:, 1, :], idf32[0:TDN, 0:TDN])
    kab_b = work.tile([OFC, 2, TDN], BF16, name="kab_b")
    nc.vector.tensor_copy(kab_b[:, 0, :], kabT_ps[:, 0, 0:TDN])
    nc.scalar.copy(kab_b[:, 1, :], kabT_ps[:, 1, 0:TDN])
    nc.scalar.copy(eeg_ext[:, 0:OFC], eeg_raw[:, :])
    # stage-2 eeg rows (off critical path; Pool engine)
    P32 = {br: work.tile([32, 32], BF16, name=f"P32_{br}") for br in ("A", "B")}
    OHR32 = {br: work.tile([32, 32], BF16, name=f"ohr32_{br}") for br in ("A", "B")}
    for t in (*P32.values(), *OHR32.values()):
        nc.gpsimd.memset(t[:, :], 0.0)
    nc.gpsimd.tensor_copy(KV[0:16, :], eeg_raw[:, :])
    nc.gpsimd.tensor_copy(dataE[0:16, :], eeg_raw[:, :])

    # eeg^T (with ones row 118) via PE transpose
    eegT_ps = pst([OFC + 1, 16], "eegT_ps", "c", dt=BF16)
    nc.tensor.transpose(eegT_ps[:, :], eeg_ext[:, :], identity[0:16, 0:16])
    eegT = work.tile([OFC + 1, 16], BF16, name="eegT")
    nc.vector.tensor_copy(eegT[:, :], eegT_ps[:, :])

    # ===================== stage 1 (A/B interleaved) ======================
    GP = {"A": cb1_sb[0:OFC, C_GPA:C_GPA+OFC+1],
          "B": cb1_sb[0:OFC, C_GPB:C_GPB+OFC+1]}
    MM = {"A": cb2_sb[0:OFC, C_MA:C_MA+OFC], "B": cb2_sb[0:OFC, C_MB:C_MB+OFC]}
    UC = {"A": cf_sb[0:OFC+1, F_UCA:F_UCA+1], "B": cf_sb[0:OFC+1, F_UCB:F_UCB+1]}
    OB16 = {"A": cf_sb[0:OFC, F_OB16A:F_OB16A+1], "B": cf_sb[0:OFC, F_OB16B:F_OB16B+1]}
    OBROW = {"A": cf_sb[0:1, F_OBROW_A:F_OBROW_A+OFC],
             "B": cf_sb[0:1, F_OBROW_B:F_OBROW_B+OFC]}
    PROJ = {"A": cb3_sb[0:1, C_PROJ:C_PROJ+16],
            "B": cb3_sb[0:1, C_PROJ+16:C_PROJ+32]}
    kT = {"A": kab_b[:, 0, :], "B": kab_b[:, 1, :]}
    tag1 = {"A": "a", "B": "b"}
    cpe = {"A": nc.vector, "B": nc.scalar}  # PSUM->SBUF copy engine per branch

    def cp(eng, out, in_):
        (eng.tensor_copy if eng is nc.vector else eng.copy)(out, in_)

    def cpadd(eng, out, in_, bias):
        if eng is nc.vector:
            eng.tensor_scalar_add(out, in_, bias)
        else:
            eng.add(out, in_, bias)

    s1 = {"A": {}, "B": {}}

    def ps1(br, shape, nm):
        return pst(shape, f"{nm}_{br}", tag1[br])

    def gk_mm(br):
        d = s1[br]
        d["gk_ps"] = ps1(br, [OFC + 1, TDN], "gk")
        nc.tensor.matmul(d["gk_ps"][:, :], GP[br], kT[br])

    def gk_post(br):
        d = s1[br]
        d["gk"] = work.tile([OFC + 1, TDN], BF16, name=f"gk_{br}")
        cpadd(cpe[br], d["gk"][:, :], d["gk_ps"][:, :], UC[br])

    def vp_mm(br):
        d = s1[br]
        d["vp_ps"] = ps1(br, [TDN, OFC], "vp")
        nc.tensor.matmul(d["vp_ps"][:, :], kT[br], MM[br])

    def vp_post(br):
        d = s1[br]
        d["vp"] = work.tile([TDN, OFC], BF16, name=f"vp_{br}")
        cp(cpe[br], d["vp"][:, :], d["vp_ps"][:, :])

    def s_mm(br):
        d = s1[br]
        d["S_ps"] = ps1(br, [16, TDN], "S")
        nc.tensor.matmul(d["S_ps"][:, :], eegT[:, :], d["gk"][:, :])

    def softmax1(br):
        d = s1[br]
        d["P"] = P32[br][0:16, 0:TDN]
        nc.scalar.activation(d["P"], d["S_ps"][:, :], AF.Exp)

    def rinv1(br):
        # off the critical path: attnT/ZT/att use unnormalized P; the
        # normalization lands on the att PSUM->SBUF copy (per-q scale)
        d = s1[br]
        d["rowsum"] = work.tile([16, 1], F32, name=f"rowsum_{br}")
        nc.vector.reduce_sum(d["rowsum"][:, :], d["P"], axis=X)
        d["rinv"] = work.tile([16, 1], F32, name=f"rinv_{br}")
        nc.vector.reciprocal(d["rinv"][:, :], d["rowsum"][:, :])

    def attnT_t(br):
        # DVE 32x32 block transpose: SBUF->SBUF, no PSUM round-trip
        d = s1[br]
        d["aT32"] = work.tile([32, 32], BF16, name=f"aT32_{br}")
        nc.vector.transpose(d["aT32"][:, :], P32[br][:, :])
        d["aT"] = d["aT32"][0:TDN, 0:16]

    def att_mm(br):
        d = s1[br]
        d["att_ps"] = ps1(br, [16, OFC], "att")
        nc.tensor.matmul(d["att_ps"][:, :], d["aT"], d["vp"][:, :])

    def att_cp(br):
        d = s1[br]
        d["att"] = work.tile([16, OFC], BF16, name=f"att_{br}")
        if br == "A":
            nc.vector.tensor_scalar_mul(d["att"][:, :], d["att_ps"][:, :],
                                        d["rinv"][:, :])
        else:
            nc.scalar.activation(d["att"][:, :], d["att_ps"][:, :], AF.Copy,
                                 scale=d["rinv"][:, :])

    def svec_mm(br):
        d = s1[br]
        d["svec_ps"] = ps1(br, [OFC, 1], "svec")
        nc.tensor.matmul(d["svec_ps"][:, :], d["att"][:, :],
                         cb3_sb[0:16, C_ONES:C_ONES+1])

    def svec_post(br):
        d = s1[br]
        d["svec"] = work.tile([OFC, 1], BF16, name=f"svec_{br}")
        cpadd(cpe[br], d["svec"][:, :], d["svec_ps"][:, :], OB16[br])

    def sc_mm(br):
        d = s1[br]
        d["sc_ps"] = ps1(br, [1, 16], "sc")
        nc.tensor.matmul(d["sc_ps"][:, :], d["svec"][:, :], eegT[0:OFC, :])

    def sel_post(br):
        d = s1[br]
        d["m"] = work.tile([1, 1], F32, name=f"m_{br}")
        nc.vector.reduce_max(d["m"][:, :], d["sc_ps"][:, :], axis=X)
        d["ohr"] = OHR32[br][0:1, 0:16]
        nc.vector.tensor_scalar(d["ohr"], d["sc_ps"][:, :], d["m"][:, :],
                                None, op0=ALU.is_equal)

    def oh_t(br):
        d = s1[br]
        d["oh32"] = work.tile([32, 32], BF16, name=f"oh32_{br}")
        nc.vector.transpose(d["oh32"][:, :], OHR32[br][:, :])
        d["oh"] = d["oh32"][0:16, 0:1]

    def row_mm(br):
        d = s1[br]
        d["row_ps"] = ps1(br, [1, OFC], "row")
        nc.tensor.matmul(d["row_ps"][:, :], d["oh"], d["att"][:, :])

    def row_post(br):
        d = s1[br]
        d["row"] = work.tile([1, OFC], BF16, name=f"row_{br}")
        nc.vector.tensor_add(d["row"][:, :], d["row_ps"][:, :], OBROW[br])

    def w_mm(br):
        d = s1[br]
        d["w_ps"] = ps1(br, [16, OFC], "w")
        nc.tensor.matmul(d["w_ps"][:, :], PROJ[br], d["row"][:, :])

    def w_cp(br):
        # wA -> DATA rows 0:16 and KV rows 16:32; wB -> DATA 51:67, KV 32:48
        d = s1[br]
        if br == "A":
            nc.vector.tensor_copy(dataA[0:16, :], d["w_ps"][:, :])
            nc.scalar.copy(KV[32:48, :], d["w_ps"][:, :])
        else:
            nc.vector.tensor_copy(dataB[0:16, :], d["w_ps"][:, :])
            nc.scalar.copy(KV[64:80, :], d["w_ps"][:, :])

    gk_mm("A")
    gk_mm("B")
    gk_post("A")
    vp_mm("A")
    gk_post("B")
    vp_mm("B")
    s_mm("A")
    vp_post("A")
    s_mm("B")
    softmax1("A")
    vp_post("B")
    rinv1("A")
    softmax1("B")
    attnT_t("A")
    rinv1("B")
    att_mm("A")
    attnT_t("B")
    att_cp("A")
    svec_mm("A")
    att_mm("B")
    svec_post("A")
    att_cp("B")
    sc_mm("A")
    svec_mm("B")
    sel_post("A")
    svec_post("B")
    oh_t("A")
    sc_mm("B")
    sel_post("B")
    row_mm("A")
    oh_t("B")
    row_post("A")
    w_mm("A")
    row_mm("B")
    w_cp("A")
    row_post("B")
    w_mm("B")
    w_cp("B")

    # ===================== stage 2 (4-way lockstep) =======================
    cpe2 = [nc.vector, nc.scalar, nc.vector, nc.scalar]
    kvb = [0, 32, 64, 0]
    hx_ps = [pst([D_CM + 1, OFC], f"hx_ps_{i}", "c" if i % 2 == 0 else "d",
                 bufs=2 if i % 2 == 0 else 1) for i in range(N_BR)]
    for i in range(N_BR):
        nc.tensor.matmul(hx_ps[i][:, :],
                         cb3_sb[kvb[i]:kvb[i]+16, C_HP+17*i:C_HP+17*i+17],
                         KV[kvb[i]:kvb[i]+16, :])
    vp2_ps = pst([OFC, 64], "vp2_ps", "d", bufs=1)
    nc.tensor.matmul(vp2_ps[:, :], KV[:, :], cb3_sb[0:80, C_M2:C_M2+64])
    hx = [work.tile([D_CM + 1, OFC], BF16, name=f"hx_{i}") for i in range(N_BR)]
    for i in range(N_BR):
        cpadd(cpe2[i], hx[i][:, :], hx_ps[i][:, :], cf_sb[0:17, F_U2+i:F_U2+i+1])
    vp2 = work.tile([OFC, 64], BF16, name="vp2")
    nc.scalar.copy(vp2[:, :], vp2_ps[:, :])

    data2 = [dataA, dataE, dataE, dataB]
    S2_ps = pst([OFC, N_BR * OFC], "S2_ps", "s2", bufs=1)
    for i in range(N_BR):
        nc.tensor.matmul(S2_ps[:, OFC*i:OFC*(i+1)], data2[i][:, :], hx[i][:, :])

    b = [dict() for _ in range(N_BR)]
    for i in range(N_BR):
        b[i]["P"] = work.tile([OFC, OFC], BF16, name=f"P2_{i}")
        nc.scalar.activation(b[i]["P"][:, :], S2_ps[:, OFC*i:OFC*(i+1)], AF.Exp)
        b[i]["rs"] = work.tile([OFC, 1], F32, name=f"rs2_{i}")
        nc.vector.reduce_sum(b[i]["rs"][:, :], b[i]["P"][:, :], axis=X)
        b[i]["rinv"] = work.tile([OFC, 1], F32, name=f"rinv2_{i}")
        nc.vector.reciprocal(b[i]["rinv"][:, :], b[i]["rs"][:, :])
        b[i]["Pn"] = work.tile([OFC, OFC], BF16, name=f"Pn2_{i}")
        if i % 2 == 0:
            nc.vector.tensor_scalar_mul(b[i]["Pn"][:, :], b[i]["P"][:, :],
                                        b[i]["rinv"][:, :])
        else:
            nc.scalar.activation(b[i]["Pn"][:, :], b[i]["P"][:, :], AF.Copy,
                                 scale=b[i]["rinv"][:, :])
        b[i]["aT_ps"] = pst([OFC, OFC], f"aT2_{i}", "a" if i % 2 == 0 else "b",
                            dt=BF16)
        nc.tensor.transpose(b[i]["aT_ps"][:, :], b[i]["Pn"][:, :],
                            identity[0:OFC, 0:OFC])
        b[i]["aT"] = work.tile([OFC, OFC], BF16, name=f"aT2_{i}")
        cp(cpe2[i], b[i]["aT"][:, :], b[i]["aT_ps"][:, :])

    for i in range(N_BR):
        b[i]["oT_ps"] = pst([D_CM, OFC], f"oT2_{i}", "c" if i % 2 == 0 else "d",
                            bufs=2 if i % 2 == 0 else 1)
        nc.tensor.matmul(b[i]["oT_ps"][:, :], vp2[:, 16*i:16*(i+1)],
                         b[i]["aT"][:, :])
        cpadd(cpe2[i], oTall[32*i:32*i+16, :], b[i]["oT_ps"][:, :],
              cf_sb[0:16, F_OB2+i:F_OB2+i+1])

    # ======================== conv + classifier ===========================
    y_ps = pst([4 * C_OUT, NCONV], "y_ps", "d", bufs=1)
    for k in range(KS):
        nc.tensor.matmul(y_ps[:, :],
                         cb3_sb[0:112, C_CONV+40*k:C_CONV+40*k+40],
                         oTall[0:112, k:k+NCONV], start=(k == 0),
                         stop=(k == KS - 1))
    ymax = work.tile([4 * C_OUT, 1], F32, name="ymax")
    nc.vector.reduce_max(ymax[:, :], y_ps[:, :], axis=X)
    feat = work.tile([4 * C_OUT, 1], BF16, name="feat")
    nc.vector.tensor_scalar(feat[:, :], ymax[:, :], cf_sb[0:40, F_CONVB:F_CONVB+1],
                            0.0, op0=ALU.add, op1=ALU.max)

    h_ps = pst([40, 1], "h_ps", "a")
    nc.tensor.matmul(h_ps[:, :], cb3_sb[0:40, C_FC1:C_FC1+40], feat[:, :])
    h = work.tile([40, 1], BF16, name="h")
    # bias holds +fc1_b here (negfb1 slot repurposed on host as +fb)
    nc.scalar.activation(h[:, :], h_ps[:, :], AF.Sigmoid,
                         bias=cf_sb[0:40, F_NFB1:F_NFB1+1], scale=1.0)

    o_ps = pst([2, 1], "o_ps", "d", bufs=1)
    nc.tensor.matmul(o_ps[:, :], cb3_sb[0:40, C_FC2:C_FC2+2], h[:, :])
    res = work.tile([2, 1], F32, name="res")
    nc.scalar.activation(res[:, :], o_ps[:, :], AF.Sigmoid,
                         bias=cf_sb[0:2, F_NFB2:F_NFB2+1], scale=1.0)

    nc.sync.dma_start(out=out_ap, in_=res[:, :])
    ctx.close()


_CACHE = {}


def build(debug_taps=False):
    key = ("nc", debug_taps)
    if key in _CACHE:
        return _CACHE[key]
    nc = bacc.Bacc("TRN2", target_bir_lowering=False, debug=False,
                   num_devices=N_CORES, num_swdge_queues=4,
                   dynamic_dma_scratch_size=65536)
    H = {name: nc.dram_tensor(name, list(shape), dt, kind="ExternalInput")
         for name, (shape, dt) in INPUT_SPECS.items()}
    out_t = nc.dram_tensor("out", [1, 2], F32, kind="ExternalOutput")
    with tile.TileContext(nc) as tc:
        _emit(nc, tc, H, out_t.ap())
    nc.compile()
    _CACHE[key] = nc
    return nc


def kernel(**inputs):
    nc = build()
    in_map = pack_inputs(inputs)
    res = run_bass_kernel_spmd(nc, [in_map] * N_CORES,
                               core_ids=list(range(N_CORES)))
    return res.results[0]["out"]
